# revision 1
# baseline (speedup 1.0000x reference)
"""CantorAttention Trainium2 kernel.

Problem (hardcoded): B=2, S=2048, DIM=512, H=8 heads, D=64, K=64 routes.
  qkv = x @ w_qkv + b_qkv ; per-head sparse attention over routes[q, :] ;
  out = attn_out @ w_out + b_out.

Strategy (8 cores): shard batch x head-pairs. Core i handles batch i//4 and
heads (2*(i%4), 2*(i%4)+1). Routes are shared across batch/heads, so the
sparse attention is run DENSE on the TensorEngine with a host-precomputed
multiplicative count-mask C^T[k, q] = #{j : routes[q, j] == k}:

  P[k, q]  = C^T[k, q] * exp(scale * (K q_vec . k_vec))       (0 off-route)
  out_h    = (V^T_aug @ P) / denom,  denom from an appended ones-column
  partial  = concat_h(out_h) @ w_out[head rows]               (per core)

Host gathers: final[b] = sum of the 4 partials of batch b + b_out.
Exactly reproduces softmax over the 64 routed scores (duplicates included
via the count mask).

Everything on PE is bf16 with fp32 PSUM accumulation; exp on ScalarE;
mask-multiply on VectorE (bf16 2x mode); transposed layouts throughout so
no on-chip transposes are needed except V (one PE transpose per key tile).
"""

import numpy as np
import ml_dtypes

import concourse.bass as bass
import concourse.bacc as bacc
import concourse.mybir as mybir
import concourse.tile as tile
from concourse.bass_utils import run_bass_kernel_spmd
from concourse.masks import make_identity

BF16 = mybir.dt.bfloat16
F32 = mybir.dt.float32
NPBF16 = ml_dtypes.bfloat16

B = 2
S = 2048
DIM = 512
H = 8
D = 64
KR = 64
SCALE = 0.125

P = 128
NKT = S // P      # 16 key tiles
QC = 512          # query chunk (psum bank width)
NQC = S // QC     # 4 query chunks
NC4 = DIM // P    # 4 contraction chunks

_CACHE = {}


def build_nc():
    if "nc" in _CACHE:
        return _CACHE["nc"]
    nc = bacc.Bacc(
        "TRN2",
        target_bir_lowering=False,
        debug=False,
        num_devices=8,
    )

    xt_d = nc.dram_tensor("xt", [P, NC4 * S], BF16, kind="ExternalInput").ap()
    wq_d = nc.dram_tensor("wq", [P, NC4 * P], BF16, kind="ExternalInput").ap()
    wk_d = nc.dram_tensor("wk", [P, NC4 * P], BF16, kind="ExternalInput").ap()
    wv_d = nc.dram_tensor("wv", [P, NC4 * P], BF16, kind="ExternalInput").ap()
    bq_d = nc.dram_tensor("bq", [P, 1], F32, kind="ExternalInput").ap()
    bk_d = nc.dram_tensor("bk", [P, 1], F32, kind="ExternalInput").ap()
    bv_d = nc.dram_tensor("bv", [P, 1], F32, kind="ExternalInput").ap()
    ct_d = nc.dram_tensor("ct", [P, NKT * S], BF16, kind="ExternalInput").ap()
    wo_d = nc.dram_tensor("wo", [P, DIM], BF16, kind="ExternalInput").ap()
    out_d = nc.dram_tensor("out", [S, DIM], F32, kind="ExternalOutput").ap()

    with tile.TileContext(nc) as tc:
        with tc.tile_pool(name="persist", bufs=1) as pp:
            ident = pp.tile([P, P], BF16, tag="ident")
            make_identity(nc, ident[:])

            xt_big = pp.tile([P, NC4 * S], BF16, tag="xtb", name="xt_big")
            nc.sync.dma_start(out=xt_big[:], in_=xt_d[:, :])
            xt_sb = [xt_big[:, c * S:(c + 1) * S] for c in range(NC4)]

            w_sb = {}
            for name, wd in (("q", wq_d), ("k", wk_d), ("v", wv_d)):
                wt = pp.tile([P, NC4 * P], BF16, tag=f"w{name}b", name=f"w{name}_big")
                nc.sync.dma_start(out=wt[:], in_=wd[:, :])
                for c in range(NC4):
                    w_sb[(name, c)] = wt[:, c * P:(c + 1) * P]
            b_sb = {}
            for name, bd in (("q", bq_d), ("k", bk_d), ("v", bv_d)):
                t = pp.tile([P, 1], F32, tag=f"b{name}", name=f"b{name}_sb")
                nc.sync.dma_start(out=t[:], in_=bd[:, :])
                b_sb[name] = t

            wo_sb = pp.tile([P, DIM], BF16, tag="wo")
            nc.sync.dma_start(out=wo_sb[:], in_=wo_d[:, :])
            sel_sb = {}
            for h in range(2):
                t = pp.tile([P, P], F32, tag=f"sel{h}", name=f"sel{h}")
                nc.vector.memset(t[:], 0.0)
                nc.vector.memset(t[0:1, h * D:(h + 1) * D], 1.0)
                sel_sb[h] = t

            ct_big = pp.tile([P, NKT * S], BF16, tag="ctb", name="ct_big")
            nc.sync.dma_start(out=ct_big[:], in_=ct_d[:, :])
            ct_sb = [ct_big[:, kt * S:(kt + 1) * S] for kt in range(NKT)]

            # v^T stacked (2 heads); q^T/k^T per-head, rows 64-127 zero-padded
            # so every main-loop matmul is a full [128,128] stationary operand.
            qkvt = {}
            qkvt["v"] = pp.tile([P, S], BF16, tag="vt", name="vt")
            for name in ("q", "k"):
                for h in range(2):
                    t = pp.tile([P, S], BF16, tag=f"{name}t{h}", name=f"{name}t{h}")
                    nc.vector.memset(t[D:P, :], 0.0)
                    qkvt[(name, h)] = t

            # Phase 1: QKV^T = W^T @ X^T (+bias), bf16.
            with tc.tile_pool(name="psum_pre", bufs=4, space="PSUM") as pre:
                for name in ("k", "q", "v"):
                    for qc in range(NQC):
                        ps = pre.tile([P, QC], F32, tag="qkvps", name="qkvps")
                        for c in range(NC4):
                            nc.tensor.matmul(
                                ps[:],
                                lhsT=w_sb[(name, c)],
                                rhs=xt_sb[c][:, qc * QC:(qc + 1) * QC],
                                start=(c == 0),
                                stop=(c == NC4 - 1),
                            )
                        if name == "v":
                            nc.vector.tensor_tensor(
                                out=qkvt["v"][:, qc * QC:(qc + 1) * QC],
                                in0=ps[:],
                                in1=b_sb["v"][:].to_broadcast([P, QC]),
                                op=mybir.AluOpType.add,
                            )
                        else:
                            for h in range(2):
                                hd = h * D
                                nc.vector.tensor_tensor(
                                    out=qkvt[(name, h)][0:D, qc * QC:(qc + 1) * QC],
                                    in0=ps[hd:hd + D, :],
                                    in1=b_sb[name][hd:hd + D, :].to_broadcast([D, QC]),
                                    op=mybir.AluOpType.add,
                                )

                # Phase 1b: V tiles in [key, d] layout with ones column.
                v_sb = {}
                for h in range(2):
                    for kt in range(NKT):
                        v_sb[(h, kt)] = pp.tile([P, P], BF16, tag=f"v{h}_{kt}", name=f"v{h}_{kt}")
                for kt in range(NKT):
                    tp = pre.tile([P, P], BF16, tag="vtps", name="vtps")
                    nc.tensor.transpose(
                        out=tp[:], in_=qkvt["v"][:, kt * P:(kt + 1) * P],
                        identity=ident[:],
                    )
                    for h in range(2):
                        nc.scalar.copy(
                            out=v_sb[(h, kt)][:, 0:D], in_=tp[:, h * D:(h + 1) * D]
                        )
                        nc.vector.memset(v_sb[(h, kt)][:, D:D + 1], 1.0)
                        nc.vector.memset(v_sb[(h, kt)][:, D + 1:P], 0.0)

            ot_sb = pp.tile([P, S], F32, tag="ot")
            den_sb = {}
            for h in range(2):
                den_sb[h] = pp.tile([P, S], F32, tag=f"den{h}", name=f"den{h}")
                nc.vector.memset(den_sb[h][D:P, :], 0.0)
                nc.vector.memset(den_sb[h][0:D, :], 0.0)
            r2r_sb = pp.tile([P, S], F32, tag="r2r")
            on_sb = pp.tile([P, S], BF16, tag="on")

            # Phase 2: dense masked attention, one head at a time.
            QH = 1024
            for h in range(2):
                hd = h * D
                with tc.tile_pool(name=f"psum_s{h}", bufs=2, space="PSUM") as sp, \
                     tc.tile_pool(name=f"psum_ot{h}", bufs=1, space="PSUM") as op, \
                     tc.tile_pool(name=f"pwork{h}", bufs=6) as pw:
                    ot_ps = op.tile([P, S], F32, tag="otps", name="otps")
                    for kt in range(NKT):
                        for q2 in range(S // QH):
                            s_ps = sp.tile([P, QH], F32, tag="s", name="s_ps")
                            for half in range(QH // QC):
                                off = q2 * QH + half * QC
                                nc.tensor.matmul(
                                    s_ps[:, half * QC:(half + 1) * QC],
                                    lhsT=qkvt[("k", h)][:, kt * P:(kt + 1) * P],
                                    rhs=qkvt[("q", h)][:, off:off + QC],
                                    start=True,
                                    stop=True,
                                )
                            p_sb = pw.tile([P, QH], BF16, tag="p", name="p_sb")
                            nc.scalar.activation(
                                p_sb[:], s_ps[:], mybir.ActivationFunctionType.Exp
                            )
                            pm_sb = pw.tile([P, QH], BF16, tag="pm", name="pm_sb")
                            nc.vector.tensor_tensor(
                                out=pm_sb[:],
                                in0=p_sb[:],
                                in1=ct_sb[kt][:, q2 * QH:(q2 + 1) * QH],
                                op=mybir.AluOpType.mult,
                            )
                            for half in range(QH // QC):
                                off = q2 * QH + half * QC
                                nc.tensor.matmul(
                                    ot_ps[:, off:off + QC],
                                    lhsT=v_sb[(h, kt)][:],
                                    rhs=pm_sb[:, half * QC:(half + 1) * QC],
                                    start=(kt == 0),
                                    stop=(kt == NKT - 1),
                                )
                    nc.scalar.copy(out=ot_sb[hd:hd + D, :], in_=ot_ps[0:D, :])
                    nc.vector.tensor_copy(out=den_sb[h][0:1, :], in_=ot_ps[D:D + 1, :])

            # Phase 3: normalize, project, store (pipelined per 512-chunk).
            with tc.tile_pool(name="psum_r2", bufs=2, space="PSUM") as rp, \
                 tc.tile_pool(name="psum_fin", bufs=3, space="PSUM") as fp, \
                 tc.tile_pool(name="fin_sb", bufs=4) as fsb:
                r2_list = []
                for qc in range(NQC):
                    qs = slice(qc * QC, (qc + 1) * QC)
                    r2_ps = rp.tile([P, QC], F32, tag="r2", name="r2_ps", bufs=4)
                    for h in range(2):
                        nc.tensor.matmul(
                            r2_ps[:],
                            lhsT=sel_sb[h][:],
                            rhs=den_sb[h][:, qs],
                            start=(h == 0),
                            stop=(h == 1),
                        )
                    r2_list.append(r2_ps)
                for qc in range(NQC):
                    qs = slice(qc * QC, (qc + 1) * QC)
                    nc.vector.reciprocal_approx_fast(out=r2r_sb[:, qs], in_=r2_list[qc][:])
                    nc.vector.tensor_tensor(
                        out=on_sb[:, qs], in0=ot_sb[:, qs], in1=r2r_sb[:, qs],
                        op=mybir.AluOpType.mult,
                    )
                    for qt in range(qc * NC4, (qc + 1) * NC4):
                        pr = fp.tile([P, DIM], F32, tag="pr", name="pr_ps")
                        nc.tensor.matmul(
                            pr[:],
                            lhsT=on_sb[:, qt * P:(qt + 1) * P],
                            rhs=wo_sb[:],
                            start=True,
                            stop=True,
                        )
                        o_sb = fsb.tile([P, DIM], F32, tag="osb", name="o_sb")
                        nc.scalar.copy(out=o_sb[:], in_=pr[:])
                        nc.sync.dma_start(
                            out=out_d[qt * P:(qt + 1) * P, :], in_=o_sb[:]
                        )

    nc.compile()
    _CACHE["nc"] = nc
    return nc


def make_in_maps(x, routes, w_qkv, b_qkv, w_out):
    x = np.asarray(x, np.float32)
    routes = np.asarray(routes)
    w_qkv = np.asarray(w_qkv, np.float32)
    b_qkv = np.asarray(b_qkv, np.float32)
    w_out = np.asarray(w_out, np.float32)

    C = np.zeros((S, S), np.float32)
    np.add.at(C, (np.arange(S)[:, None], routes), 1.0)

    def pack(a):
        # [n*128, X] -> [128, n*X]
        n = a.shape[0] // P
        return np.ascontiguousarray(
            a.reshape(n, P, a.shape[1]).transpose(1, 0, 2).reshape(P, -1))

    xt = [pack(np.ascontiguousarray(x[b].T)).astype(NPBF16) for b in range(B)]
    ctp = pack(np.ascontiguousarray(C.T)).astype(NPBF16)

    in_maps = []
    for core in range(8):
        b = core // 4
        hp = core % 4
        col = hp * P
        wq = pack(w_qkv[:, col:col + P] * SCALE).astype(NPBF16)
        wk = pack(w_qkv[:, DIM + col:DIM + col + P]).astype(NPBF16)
        wv = pack(w_qkv[:, 2 * DIM + col:2 * DIM + col + P]).astype(NPBF16)
        bq = (b_qkv[col:col + P] * SCALE).astype(np.float32).reshape(P, 1)
        bk = b_qkv[DIM + col:DIM + col + P].astype(np.float32).reshape(P, 1)
        bv = b_qkv[2 * DIM + col:2 * DIM + col + P].astype(np.float32).reshape(P, 1)
        wo = np.ascontiguousarray(w_out[col:col + P, :]).astype(NPBF16)
        in_maps.append(dict(
            xt=xt[b], wq=wq, wk=wk, wv=wv, bq=bq, bk=bk, bv=bv,
            ct=ctp, wo=wo,
        ))
    return in_maps


def run(inputs, trace=False, trace_cores=None):
    nc = build_nc()
    in_maps = make_in_maps(
        inputs["x"], inputs["routes"], inputs["w_qkv"], inputs["b_qkv"],
        inputs["w_out"],
    )
    res = run_bass_kernel_spmd(
        nc, in_maps, list(range(8)), trace=trace, trace_cores=trace_cores,
    )
    b_out = np.asarray(inputs["b_out"], np.float32)
    final = np.zeros((B, S, DIM), np.float32)
    for core in range(8):
        final[core // 4] += res.results[core]["out"]
    final += b_out[None, None, :]
    return final, res


def kernel(**inputs):
    final, _ = run(inputs, trace=False)
    return final



# revision 11
# speedup vs baseline: 1.4859x; 1.4859x over previous
"""CantorAttention Trainium2 kernel — block-sparse banded attention.

Problem (hardcoded): B=2, S=2048, DIM=512, H=8 heads, D=64, K=64 routes.
  qkv = x @ w_qkv + b_qkv ; per-head sparse attention over routes[q, :] ;
  out = attn_out @ w_out + b_out.

Sharding (8 cores): core i handles batch i//4, heads (2*(i%4), 2*(i%4)+1).
Host gathers: final[b] = sum of the 4 partials of batch b + b_out.

Key idea: routes are k-NN in Cantor-coordinate space. A spectral
seriation of the route graph (host-side) finds a permutation of
positions under which the route matrix is a narrow band: every 128-query
tile's routes fall in a ~229-key window => 2 unaligned 128-key slices.
Attention is computed DENSE per (qtile, slice) block with a
multiplicative count-mask (exact softmax semantics, duplicates
included), skipping everything outside the band: ~5.6x less score/PV/
exp work than full dense.

Softmax denominator: V_aug = [V | ones-row] transposed per slice, so the
PV matmul's row 64 accumulates sum_k pm[k,q] = denominator. Normalize
with a partition-broadcast reciprocal+mult straight out of PSUM.

k-bias is dropped entirely (softmax is invariant to per-query score
shifts); q-bias and the 1/sqrt(D) scale are folded host-side into wq/bq.
"""

import numpy as np
import ml_dtypes

import concourse.bass as bass
import concourse.bacc as bacc
import concourse.mybir as mybir
import concourse.tile as tile
from concourse.bass_utils import run_bass_kernel_spmd
from concourse.masks import make_identity

BF16 = mybir.dt.bfloat16
F32 = mybir.dt.float32
NPBF16 = ml_dtypes.bfloat16

B = 2
S = 2048
DIM = 512
H = 8
D = 64
KR = 64
SCALE = 0.125

P = 128
NQT = S // P      # 16 query tiles
NC4 = DIM // P    # 4 contraction chunks
QC = 512          # phase-1 column chunk

# Partition-broadcast APs are rejected by the AP checker (partition step
# must be nonzero), so replicate denominators via a small matmul instead.
USE_PART_BCAST = False

_CACHE = {}


def _plan_windows(routes):
    """Host: permutation + per-qtile key-slice offsets from routes alone."""
    routes = np.asarray(routes)
    s = routes.shape[0]
    x = np.arange(s, dtype=np.float64)
    for _ in range(60):
        x = x[routes].mean(1)
        x -= x.mean()
        n = np.linalg.norm(x)
        if n > 0:
            x /= n
    perm = np.argsort(x, kind="stable").astype(np.int64)
    inv = np.empty(s, np.int64)
    inv[perm] = np.arange(s)
    rk = inv[routes[perm]]  # routes in sorted space
    slices = []
    for t in range(s // P):
        r = rk[t * P:(t + 1) * P]
        lo, hi = int(r.min()), int(r.max())
        n_sl = max(2, int(np.ceil((hi - lo + 1) / P)))
        w0 = min(max(0, lo), s - n_sl * P)
        slices.append([w0 + j * P for j in range(n_sl)])
    return perm, inv, rk, slices


def build_nc(slices):
    key = tuple(tuple(s) for s in slices)
    if key in _CACHE:
        return _CACHE[key]
    nsl = [len(s) for s in slices]          # slices per qtile (>=2)
    tot_sl = sum(nsl)                       # total slice count
    sl_base = np.cumsum([0] + nsl).tolist() # block index base per qtile

    nc = bacc.Bacc(
        "TRN2",
        target_bir_lowering=False,
        debug=False,
        num_devices=8,
    )

    xt_d = nc.dram_tensor("xt", [P, NC4 * S], BF16, kind="ExternalInput").ap()
    wq_d = nc.dram_tensor("wq", [P, NC4 * P], BF16, kind="ExternalInput").ap()
    wk_d = nc.dram_tensor("wk", [P, NC4 * P], BF16, kind="ExternalInput").ap()
    wv_d = nc.dram_tensor("wv", [P, NC4 * P], BF16, kind="ExternalInput").ap()
    bq_d = nc.dram_tensor("bq", [P, 1], F32, kind="ExternalInput").ap()
    bv_d = nc.dram_tensor("bv", [P, 1], F32, kind="ExternalInput").ap()
    # mask columns: per qtile, per slice, per head: [128k, 128q] blocks,
    # duplicated for the 2 heads: layout [h0s0|h0s1|...|h1s0|h1s1|...]
    msk_d = nc.dram_tensor("msk", [P, 2 * tot_sl * P], BF16,
                           kind="ExternalInput").ap()
    wo_d = nc.dram_tensor("wo", [P, DIM], BF16, kind="ExternalInput").ap()
    out_d = nc.dram_tensor("out", [S, DIM], BF16, kind="ExternalOutput").ap()

    with tile.TileContext(nc) as tc:
        with tc.tile_pool(name="persist", bufs=1) as pp:
            ident = pp.tile([P, P], BF16, tag="ident")
            make_identity(nc, ident[:])

            xt_sb = pp.tile([P, NC4 * S], BF16, tag="xt")
            nc.sync.dma_start(out=xt_sb[:], in_=xt_d[:, :])

            w_sb = {}
            for name, wd in (("q", wq_d), ("k", wk_d), ("v", wv_d)):
                t = pp.tile([P, NC4 * P], BF16, tag=f"w{name}")
                nc.sync.dma_start(out=t[:], in_=wd[:, :])
                w_sb[name] = t
            bq_sb = pp.tile([P, 1], F32, tag="bq")
            nc.sync.dma_start(out=bq_sb[:], in_=bq_d[:, :])
            bv_sb = pp.tile([P, 1], F32, tag="bv")
            nc.sync.dma_start(out=bv_sb[:], in_=bv_d[:, :])
            wo_sb = pp.tile([P, DIM], BF16, tag="wo")
            nc.sync.dma_start(out=wo_sb[:], in_=wo_d[:, :])
            msk_sb = pp.tile([P, 2 * tot_sl * P], BF16, tag="msk")
            nc.sync.dma_start(out=msk_sb[:], in_=msk_d[:, :])

            # q^T/k^T per head, rows 64-127 zero-padded so every score
            # matmul is a full 128-contraction base-0 operand (HW-safe).
            qT = [pp.tile([P, S], BF16, tag=f"qT{h}", name=f"qT{h}")
                  for h in range(2)]
            kT = [pp.tile([P, S], BF16, tag=f"kT{h}", name=f"kT{h}")
                  for h in range(2)]
            for h in range(2):
                nc.gpsimd.memset(qT[h][D:P, :], 0.0)
                nc.gpsimd.memset(kT[h][D:P, :], 0.0)
            vT = pp.tile([P, S], BF16, tag="vT")

            # V_aug per (qtile, slice, head): [128k, 65] padded to 66 cols
            # (PSUM/bf16 offsets must be 4-byte aligned); col 64 of each
            # block = ones (denominator row), set once via strided memset.
            VA = D + 2
            VAUG = pp.tile([P, 2 * tot_sl * VA], BF16, tag="vaug")
            nc.gpsimd.memset(
                VAUG[:].rearrange("p (b va) -> p b va", va=VA)[:, :, D:D + 1],
                1.0)
            ON = pp.tile([P, S], BF16, tag="on")    # normalized attn, hidden-major
            if not USE_PART_BCAST:
                SEL = pp.tile([D + 1, D], BF16, tag="sel")
                nc.vector.memset(SEL[0:D, :], 0.0)
                nc.vector.memset(SEL[D:D + 1, :], 1.0)
                OTS = pp.tile([D + 1, NQT * 2 * P], BF16, tag="ots")

            # ---- Phase 1: QKV^T = W^T @ X^T (k needs no bias) ----
            with tc.tile_pool(name="ph1", bufs=4, space="PSUM") as ph1:
                for qc in range(NC4):
                    cs = slice(qc * QC, (qc + 1) * QC)
                    for name in ("k", "q", "v"):
                        ps = ph1.tile([P, QC], F32, tag="qkv", name="qkv_ps")
                        for c in range(NC4):
                            nc.tensor.matmul(
                                ps[:],
                                lhsT=w_sb[name][:, c * P:(c + 1) * P],
                                rhs=xt_sb[:, c * S + qc * QC:
                                          c * S + (qc + 1) * QC],
                                start=(c == 0),
                                stop=(c == NC4 - 1),
                            )
                        if name == "q":
                            for h in range(2):
                                hd = h * D
                                nc.scalar.activation(
                                    qT[h][0:D, cs], ps[hd:hd + D, :],
                                    mybir.ActivationFunctionType.Identity,
                                    bias=bq_sb[hd:hd + D, :],
                                )
                        elif name == "k":
                            for h in range(2):
                                hd = h * D
                                nc.vector.tensor_copy(
                                    out=kT[h][0:D, cs], in_=ps[hd:hd + D, :])
                        else:
                            nc.scalar.activation(
                                vT[:, cs], ps[:],
                                mybir.ActivationFunctionType.Identity,
                                bias=bv_sb[:],
                            )

            # ---- Phase 2: banded attention ----
            with tc.tile_pool(name="vtp", bufs=1, space="PSUM") as vtp, \
                 tc.tile_pool(name="sp", bufs=3, space="PSUM") as sp, \
                 tc.tile_pool(name="otp", bufs=2, space="PSUM") as otp, \
                 tc.tile_pool(name="prp", bufs=2, space="PSUM") as prp, \
                 tc.tile_pool(name="pmp", bufs=3) as pmp, \
                 tc.tile_pool(name="obp", bufs=3) as obp:
                for t in range(NQT):
                    sl = slices[t]
                    ns = len(sl)
                    qs = slice(t * P, (t + 1) * P)
                    base = sl_base[t]

                    # V transposes: full [128,128] per slice (both heads)
                    vt_ps = vtp.tile([P, ns * P], BF16, tag="vt",
                                     name="vt_ps")
                    for j, w in enumerate(sl):
                        nc.tensor.transpose(
                            out=vt_ps[:, j * P:(j + 1) * P],
                            in_=vT[:, w:w + P],
                            identity=ident[:],
                        )
                    vg0 = 2 * base * VA
                    nc.vector.tensor_copy(
                        out=VAUG[:, vg0:vg0 + 2 * ns * VA].rearrange(
                            "p (b va) -> p b va", va=VA)[:, :, 0:D],
                        in_=vt_ps[:].rearrange(
                            "p (b d) -> p b d", d=D))

                    # scores: per (head, slice) [128k, 128q]
                    sc = sp.tile([P, 2 * ns * P], F32, tag="s", name="s_ps")
                    for h in range(2):
                        for j, w in enumerate(sl):
                            col = (ns * h + j) * P
                            nc.tensor.matmul(
                                sc[:, col:col + P],
                                lhsT=kT[h][:, w:w + P],
                                rhs=qT[h][:, qs],
                                start=True,
                                stop=True,
                            )
                    # exp then in-place count-mask multiply
                    pm = pmp.tile([P, 2 * ns * P], BF16, tag="pm", name="pm_sb")
                    nc.scalar.activation(
                        pm[:], sc[:], mybir.ActivationFunctionType.Exp)
                    mcol = 2 * base * P
                    nc.vector.tensor_tensor(
                        out=pm[:], in0=pm[:],
                        in1=msk_sb[:, mcol:mcol + 2 * ns * P],
                        op=mybir.AluOpType.mult,
                    )

                    # PV: accumulate [65, 128] per head; key-halves split so
                    # 64-row stationaries alternate PE quadrants.
                    ot = otp.tile([P, 2 * P], F32, tag="ot", name="ot_ps")
                    for h in range(2):
                        oc = h * P
                        for j in range(ns):
                            va = vg0 + (2 * j + h) * VA
                            pc = (ns * h + j) * P
                            nc.tensor.matmul(
                                ot[0:D + 1, oc:oc + P],
                                lhsT=VAUG[:, va:va + D + 1],
                                rhs=pm[:, pc:pc + P],
                                start=(j == 0),
                                stop=(j == ns - 1),
                            )

                    # normalize: on = ot / den (den = row 64), partition-bcast
                    if USE_PART_BCAST:
                        for h in range(2):
                            oc = h * P
                            rec = pmp.tile([1, P], F32, tag="rec", name="rec")
                            nc.vector.reciprocal_approx_fast(
                                out=rec[:], in_=ot[D:D + 1, oc:oc + P])
                            nc.vector.tensor_tensor(
                                out=ON[h * D:(h + 1) * D, qs],
                                in0=ot[0:D, oc:oc + P],
                                in1=rec[0:1, :].to_broadcast([D, P]),
                                op=mybir.AluOpType.mult,
                            )
                    else:
                        oc0 = t * 2 * P
                        nc.scalar.copy(
                            out=OTS[:, oc0:oc0 + 2 * P], in_=ot[0:D + 1, :])
                        r2 = sp.tile([P, 2 * ns * P], F32, tag="s", name="r2")
                        nc.tensor.matmul(
                            r2[0:D, 0:2 * P],
                            lhsT=SEL[:],
                            rhs=OTS[:, oc0:oc0 + 2 * P],
                            start=True, stop=True,
                        )
                        rr = pmp.tile([D, 2 * P], F32, tag="rr", name="rr")
                        nc.vector.reciprocal_approx_fast(
                            out=rr[:], in_=r2[0:D, 0:2 * P])
                        for h in range(2):
                            nc.vector.tensor_tensor(
                                out=ON[h * D:(h + 1) * D, qs],
                                in0=OTS[0:D, oc0 + h * P:oc0 + (h + 1) * P],
                                in1=rr[:, h * P:(h + 1) * P],
                                op=mybir.AluOpType.mult,
                            )

                    # project + store
                    pr = prp.tile([P, DIM], F32, tag="pr", name="pr_ps")
                    nc.tensor.matmul(
                        pr[:], lhsT=ON[:, qs], rhs=wo_sb[:],
                        start=True, stop=True,
                    )
                    ob = obp.tile([P, DIM], BF16, tag="ob", name="ob_sb")
                    if t % 2 == 0:
                        nc.scalar.copy(out=ob[:], in_=pr[:])
                    else:
                        nc.vector.tensor_copy(out=ob[:], in_=pr[:])
                    nc.sync.dma_start(out=out_d[qs, :], in_=ob[:])

    nc.compile()
    _CACHE[key] = nc
    return nc


def _pack(a):
    # [n*128, X] -> [128, n*X] grouping row-blocks along columns
    n = a.shape[0] // P
    return np.ascontiguousarray(
        a.reshape(n, P, a.shape[1]).transpose(1, 0, 2).reshape(P, -1))


def make_in_maps(x, routes, w_qkv, b_qkv, w_out):
    x = np.asarray(x, np.float32)
    routes = np.asarray(routes)
    w_qkv = np.asarray(w_qkv, np.float32)
    b_qkv = np.asarray(b_qkv, np.float32)
    w_out = np.asarray(w_out, np.float32)

    perm, inv, rk, slices = _plan_windows(routes)

    # count-mask blocks in permuted space: C~[k, q]
    Ct = np.zeros((S, S), np.float32)
    np.add.at(Ct, (rk.ravel(),
                   np.repeat(np.arange(S), KR)), 1.0)
    msk_cols = []
    for t, sl in enumerate(slices):
        for h in range(2):
            for w in sl:
                msk_cols.append(Ct[w:w + P, t * P:(t + 1) * P])
    msk = np.concatenate(msk_cols, axis=1).astype(NPBF16)
    msk = np.ascontiguousarray(msk)

    xt = [_pack(np.ascontiguousarray(x[b][perm].T)).astype(NPBF16)
          for b in range(B)]

    in_maps = []
    for core in range(8):
        b = core // 4
        hp = core % 4
        col = hp * P
        wq = _pack(w_qkv[:, col:col + P] * SCALE).astype(NPBF16)
        wk = _pack(w_qkv[:, DIM + col:DIM + col + P]).astype(NPBF16)
        wv = _pack(w_qkv[:, 2 * DIM + col:2 * DIM + col + P]).astype(NPBF16)
        bq = (b_qkv[col:col + P] * SCALE).astype(np.float32).reshape(P, 1)
        bv = b_qkv[2 * DIM + col:2 * DIM + col + P].astype(
            np.float32).reshape(P, 1)
        wo = np.ascontiguousarray(w_out[col:col + P, :]).astype(NPBF16)
        in_maps.append(dict(
            xt=xt[b], wq=wq, wk=wk, wv=wv, bq=bq, bv=bv, msk=msk, wo=wo,
        ))
    return in_maps, perm, slices


def run(inputs, trace=False, trace_cores=None):
    in_maps, perm, slices = make_in_maps(
        inputs["x"], inputs["routes"], inputs["w_qkv"], inputs["b_qkv"],
        inputs["w_out"],
    )
    nc = build_nc(slices)
    res = run_bass_kernel_spmd(
        nc, in_maps, list(range(8)), trace=trace, trace_cores=trace_cores,
    )
    b_out = np.asarray(inputs["b_out"], np.float32)
    final = np.zeros((B, S, DIM), np.float32)
    for core in range(8):
        final[core // 4][perm] += np.asarray(
            res.results[core]["out"], np.float32)
    final += b_out[None, None, :]
    return final, res


def kernel(**inputs):
    final, _ = run(inputs, trace=False)
    return final


# revision 13
# speedup vs baseline: 1.8673x; 1.2567x over previous
"""CantorAttention Trainium2 kernel — block-sparse banded attention.

Problem (hardcoded): B=2, S=2048, DIM=512, H=8 heads, D=64, K=64 routes.
  qkv = x @ w_qkv + b_qkv ; per-head sparse attention over routes[q, :] ;
  out = attn_out @ w_out + b_out.

Sharding (8 cores): core i handles batch i//4, heads (2*(i%4), 2*(i%4)+1).
Host gathers: final[b] = sum of the 4 partials of batch b + b_out.

Key idea: routes are k-NN in Cantor-coordinate space. A spectral
seriation of the route graph (host-side) finds a permutation of
positions under which the route matrix is a narrow band: every 128-query
tile's routes fall in a ~229-key window => 2 unaligned 128-key slices.
Attention is computed DENSE per (qtile, slice) block with a
multiplicative count-mask (exact softmax semantics, duplicates
included), skipping everything outside the band: ~5.6x less score/PV/
exp work than full dense.

Softmax denominator: V_aug = [V | ones-row] transposed per slice, so the
PV matmul's row 64 accumulates sum_k pm[k,q] = denominator. Normalize
with a partition-broadcast reciprocal+mult straight out of PSUM.

k-bias is dropped entirely (softmax is invariant to per-query score
shifts); q-bias and the 1/sqrt(D) scale are folded host-side into wq/bq.
"""

import numpy as np
import ml_dtypes

import concourse.bass as bass
import concourse.bacc as bacc
import concourse.mybir as mybir
import concourse.tile as tile
from concourse.bass_utils import run_bass_kernel_spmd
from concourse.masks import make_identity

BF16 = mybir.dt.bfloat16
F32 = mybir.dt.float32
NPBF16 = ml_dtypes.bfloat16

B = 2
S = 2048
DIM = 512
H = 8
D = 64
KR = 64
SCALE = 0.125

P = 128
NQT = S // P      # 16 query tiles
NC4 = DIM // P    # 4 contraction chunks
QC = 512          # phase-1 column chunk

# Partition-broadcast APs are rejected by the AP checker (partition step
# must be nonzero), so replicate denominators via a small matmul instead.
USE_PART_BCAST = False

_CACHE = {}


def _plan_windows(routes):
    """Host: permutation + per-qtile key-slice offsets from routes alone."""
    routes = np.asarray(routes)
    s = routes.shape[0]
    x = np.arange(s, dtype=np.float64)
    for _ in range(60):
        x = x[routes].mean(1)
        x -= x.mean()
        n = np.linalg.norm(x)
        if n > 0:
            x /= n
    perm = np.argsort(x, kind="stable").astype(np.int64)
    inv = np.empty(s, np.int64)
    inv[perm] = np.arange(s)
    rk = inv[routes[perm]]  # routes in sorted space
    slices = []
    for t in range(s // P):
        r = rk[t * P:(t + 1) * P]
        lo, hi = int(r.min()), int(r.max())
        n_sl = max(2, int(np.ceil((hi - lo + 1) / P)))
        w0 = min(max(0, lo), s - n_sl * P)
        slices.append([w0 + j * P for j in range(n_sl)])
    return perm, inv, rk, slices


def build_nc(slices):
    key = tuple(tuple(s) for s in slices)
    if key in _CACHE:
        return _CACHE[key]
    nsl = [len(s) for s in slices]          # slices per qtile (>=2)
    tot_sl = sum(nsl)                       # total slice count
    sl_base = np.cumsum([0] + nsl).tolist() # block index base per qtile

    nc = bacc.Bacc(
        "TRN2",
        target_bir_lowering=False,
        debug=False,
        num_devices=8,
    )

    xt_d = nc.dram_tensor("xt", [P, NC4 * S], BF16, kind="ExternalInput").ap()
    wq_d = nc.dram_tensor("wq", [P, NC4 * P], BF16, kind="ExternalInput").ap()
    wk_d = nc.dram_tensor("wk", [P, NC4 * P], BF16, kind="ExternalInput").ap()
    wv_d = nc.dram_tensor("wv", [P, NC4 * P], BF16, kind="ExternalInput").ap()
    bq_d = nc.dram_tensor("bq", [P, 1], F32, kind="ExternalInput").ap()
    bv_d = nc.dram_tensor("bv", [P, 1], F32, kind="ExternalInput").ap()
    # mask columns: per qtile, per slice, per head: [128k, 128q] blocks,
    # duplicated for the 2 heads: layout [h0s0|h0s1|...|h1s0|h1s1|...]
    msk_d = nc.dram_tensor("msk", [P, 2 * tot_sl * P], BF16,
                           kind="ExternalInput").ap()
    wo_d = nc.dram_tensor("wo", [P, DIM], BF16, kind="ExternalInput").ap()
    out_d = nc.dram_tensor("out", [S, DIM], BF16, kind="ExternalOutput").ap()

    with tile.TileContext(nc) as tc:
        with tc.tile_pool(name="persist", bufs=1) as pp:
            ident = pp.tile([P, P], BF16, tag="ident")
            make_identity(nc, ident[:])

            xt_sb = pp.tile([P, NC4 * S], BF16, tag="xt")
            for qc in range(NC4):
                nc.sync.dma_start(
                    out=xt_sb[:].rearrange("p (c s) -> p c s", c=NC4)[
                        :, :, qc * QC:(qc + 1) * QC],
                    in_=xt_d[:, :].rearrange("p (c s) -> p c s", c=NC4)[
                        :, :, qc * QC:(qc + 1) * QC])

            w_sb = {}
            for name, wd in (("q", wq_d), ("k", wk_d), ("v", wv_d)):
                t = pp.tile([P, NC4 * P], BF16, tag=f"w{name}")
                nc.sync.dma_start(out=t[:], in_=wd[:, :])
                w_sb[name] = t
            bq_sb = pp.tile([P, 1], F32, tag="bq")
            nc.sync.dma_start(out=bq_sb[:], in_=bq_d[:, :])
            bv_sb = pp.tile([P, 1], F32, tag="bv")
            nc.sync.dma_start(out=bv_sb[:], in_=bv_d[:, :])
            wo_sb = pp.tile([P, DIM], BF16, tag="wo")
            nc.sync.dma_start(out=wo_sb[:], in_=wo_d[:, :])
            msk_sb = pp.tile([P, 2 * tot_sl * P], BF16, tag="msk")
            mw = 2 * tot_sl * P
            mstep = -(-mw // (4 * P)) * P
            for mo in range(0, mw, mstep):
                me = min(mo + mstep, mw)
                nc.sync.dma_start(out=msk_sb[:, mo:me], in_=msk_d[:, mo:me])

            # q^T/k^T per head, rows 64-127 zero-padded so every score
            # matmul is a full 128-contraction base-0 operand (HW-safe).
            qT = [pp.tile([P, S], BF16, tag=f"qT{h}", name=f"qT{h}")
                  for h in range(2)]
            kT = [pp.tile([P, S], BF16, tag=f"kT{h}", name=f"kT{h}")
                  for h in range(2)]
            for h in range(2):
                nc.gpsimd.memset(qT[h][D:P, :], 0.0)
                nc.gpsimd.memset(kT[h][D:P, :], 0.0)
            vT = pp.tile([P, S], BF16, tag="vT")

            # V_aug per (qtile, slice, head): [128k, 65] padded to 66 cols
            # (PSUM/bf16 offsets must be 4-byte aligned); col 64 of each
            # block = ones (denominator row), set once via strided memset.
            VA = D + 2
            VAUG = pp.tile([P, 2 * tot_sl * VA], BF16, tag="vaug")
            nc.gpsimd.memset(
                VAUG[:].rearrange("p (b va) -> p b va", va=VA)[:, :, D:D + 1],
                1.0)
            ON = pp.tile([P, S], BF16, tag="on")    # normalized attn, hidden-major
            if not USE_PART_BCAST:
                SEL = pp.tile([D + 1, D], BF16, tag="sel")
                nc.vector.memset(SEL[0:D, :], 0.0)
                nc.vector.memset(SEL[D:D + 1, :], 1.0)
                OTS = pp.tile([D + 1, NQT * 2 * P], BF16, tag="ots")

            # ---- Phase 1: QKV^T = W^T @ X^T (k needs no bias) ----
            with tc.tile_pool(name="ph1", bufs=4, space="PSUM") as ph1:
                for qc in range(NC4):
                    cs = slice(qc * QC, (qc + 1) * QC)
                    for name in ("k", "q", "v"):
                        ps = ph1.tile([P, QC], F32, tag="qkv", name="qkv_ps")
                        for c in range(NC4):
                            nc.tensor.matmul(
                                ps[:],
                                lhsT=w_sb[name][:, c * P:(c + 1) * P],
                                rhs=xt_sb[:, c * S + qc * QC:
                                          c * S + (qc + 1) * QC],
                                start=(c == 0),
                                stop=(c == NC4 - 1),
                            )
                        if name == "q":
                            for h in range(2):
                                hd = h * D
                                nc.scalar.activation(
                                    qT[h][0:D, cs], ps[hd:hd + D, :],
                                    mybir.ActivationFunctionType.Identity,
                                    bias=bq_sb[hd:hd + D, :],
                                )
                        elif name == "k":
                            for h in range(2):
                                hd = h * D
                                nc.vector.tensor_copy(
                                    out=kT[h][0:D, cs], in_=ps[hd:hd + D, :])
                        else:
                            nc.scalar.activation(
                                vT[:, cs], ps[:],
                                mybir.ActivationFunctionType.Identity,
                                bias=bv_sb[:],
                            )

            # ---- Phase 2: banded attention ----
            with tc.tile_pool(name="vtp", bufs=1, space="PSUM") as vtp, \
                 tc.tile_pool(name="sp", bufs=3, space="PSUM") as sp, \
                 tc.tile_pool(name="otp", bufs=2, space="PSUM") as otp, \
                 tc.tile_pool(name="prp", bufs=2, space="PSUM") as prp, \
                 tc.tile_pool(name="pmp", bufs=3) as pmp, \
                 tc.tile_pool(name="obp", bufs=3) as obp:
                state = {}

                def stage_front(t):
                    sl = slices[t]
                    ns = len(sl)
                    qs = slice(t * P, (t + 1) * P)
                    base = sl_base[t]

                    # V transposes: full [128,128] per slice (both heads)
                    vt_ps = vtp.tile([P, ns * P], BF16, tag="vt",
                                     name="vt_ps")
                    for j, w in enumerate(sl):
                        nc.tensor.transpose(
                            out=vt_ps[:, j * P:(j + 1) * P],
                            in_=vT[:, w:w + P],
                            identity=ident[:],
                        )
                    vg0 = 2 * base * VA
                    nc.vector.tensor_copy(
                        out=VAUG[:, vg0:vg0 + 2 * ns * VA].rearrange(
                            "p (b va) -> p b va", va=VA)[:, :, 0:D],
                        in_=vt_ps[:].rearrange(
                            "p (b d) -> p b d", d=D))

                    # scores: per (head, slice) [128k, 128q]
                    sc = sp.tile([P, 2 * ns * P], F32, tag="s", name="s_ps")
                    for h in range(2):
                        for j, w in enumerate(sl):
                            col = (ns * h + j) * P
                            nc.tensor.matmul(
                                sc[:, col:col + P],
                                lhsT=kT[h][:, w:w + P],
                                rhs=qT[h][:, qs],
                                start=True,
                                stop=True,
                            )
                    # exp then in-place count-mask multiply
                    pm = pmp.tile([P, 2 * ns * P], BF16, tag="pm", name="pm_sb")
                    nc.scalar.activation(
                        pm[:], sc[:], mybir.ActivationFunctionType.Exp)
                    mcol = 2 * base * P
                    nc.vector.tensor_tensor(
                        out=pm[:], in0=pm[:],
                        in1=msk_sb[:, mcol:mcol + 2 * ns * P],
                        op=mybir.AluOpType.mult,
                    )

                    state[t] = (pm, vg0)

                def stage_back(t):
                    sl = slices[t]
                    ns = len(sl)
                    qs = slice(t * P, (t + 1) * P)
                    pm, vg0 = state.pop(t)

                    # PV: accumulate [65, 128] per head
                    ot = otp.tile([P, 2 * P], F32, tag="ot", name="ot_ps")
                    for h in range(2):
                        oc = h * P
                        for j in range(ns):
                            va = vg0 + (2 * j + h) * VA
                            pc = (ns * h + j) * P
                            nc.tensor.matmul(
                                ot[0:D + 1, oc:oc + P],
                                lhsT=VAUG[:, va:va + D + 1],
                                rhs=pm[:, pc:pc + P],
                                start=(j == 0),
                                stop=(j == ns - 1),
                            )

                    # normalize: on = ot / den (den = row 64), partition-bcast
                    if USE_PART_BCAST:
                        for h in range(2):
                            oc = h * P
                            rec = pmp.tile([1, P], F32, tag="rec", name="rec")
                            nc.vector.reciprocal_approx_fast(
                                out=rec[:], in_=ot[D:D + 1, oc:oc + P])
                            nc.vector.tensor_tensor(
                                out=ON[h * D:(h + 1) * D, qs],
                                in0=ot[0:D, oc:oc + P],
                                in1=rec[0:1, :].to_broadcast([D, P]),
                                op=mybir.AluOpType.mult,
                            )
                    else:
                        oc0 = t * 2 * P
                        nc.scalar.copy(
                            out=OTS[:, oc0:oc0 + 2 * P], in_=ot[0:D + 1, :])
                        r2 = sp.tile([P, 2 * ns * P], F32, tag="s", name="r2")
                        nc.tensor.matmul(
                            r2[0:D, 0:2 * P],
                            lhsT=SEL[:],
                            rhs=OTS[:, oc0:oc0 + 2 * P],
                            start=True, stop=True,
                        )
                        rr = pmp.tile([D, 2 * P], F32, tag="rr", name="rr")
                        nc.vector.reciprocal_approx_fast(
                            out=rr[:], in_=r2[0:D, 0:2 * P])
                        for h in range(2):
                            nc.vector.tensor_tensor(
                                out=ON[h * D:(h + 1) * D, qs],
                                in0=OTS[0:D, oc0 + h * P:oc0 + (h + 1) * P],
                                in1=rr[:, h * P:(h + 1) * P],
                                op=mybir.AluOpType.mult,
                            )

                    # project + store
                    pr = prp.tile([P, DIM], F32, tag="pr", name="pr_ps")
                    nc.tensor.matmul(
                        pr[:], lhsT=ON[:, qs], rhs=wo_sb[:],
                        start=True, stop=True,
                    )
                    ob = obp.tile([P, DIM], BF16, tag="ob", name="ob_sb")
                    if t % 2 == 0:
                        nc.scalar.copy(out=ob[:], in_=pr[:])
                    else:
                        nc.vector.tensor_copy(out=ob[:], in_=pr[:])
                    nc.sync.dma_start(out=out_d[qs, :], in_=ob[:])

                # software-pipelined emission: front of tile t+1 is queued
                # before the back of tile t so PE never drains waiting on
                # the Act/DVE exp->mask chain.
                stage_front(0)
                for t in range(1, NQT):
                    stage_front(t)
                    stage_back(t - 1)
                stage_back(NQT - 1)

    nc.compile()
    _CACHE[key] = nc
    return nc


def _pack(a):
    # [n*128, X] -> [128, n*X] grouping row-blocks along columns
    n = a.shape[0] // P
    return np.ascontiguousarray(
        a.reshape(n, P, a.shape[1]).transpose(1, 0, 2).reshape(P, -1))


def make_in_maps(x, routes, w_qkv, b_qkv, w_out):
    x = np.asarray(x, np.float32)
    routes = np.asarray(routes)
    w_qkv = np.asarray(w_qkv, np.float32)
    b_qkv = np.asarray(b_qkv, np.float32)
    w_out = np.asarray(w_out, np.float32)

    perm, inv, rk, slices = _plan_windows(routes)

    # count-mask blocks in permuted space: C~[k, q]
    Ct = np.zeros((S, S), np.float32)
    np.add.at(Ct, (rk.ravel(),
                   np.repeat(np.arange(S), KR)), 1.0)
    msk_cols = []
    for t, sl in enumerate(slices):
        for h in range(2):
            for w in sl:
                msk_cols.append(Ct[w:w + P, t * P:(t + 1) * P])
    msk = np.concatenate(msk_cols, axis=1).astype(NPBF16)
    msk = np.ascontiguousarray(msk)

    xt = [_pack(np.ascontiguousarray(x[b][perm].T)).astype(NPBF16)
          for b in range(B)]

    in_maps = []
    for core in range(8):
        b = core // 4
        hp = core % 4
        col = hp * P
        wq = _pack(w_qkv[:, col:col + P] * SCALE).astype(NPBF16)
        wk = _pack(w_qkv[:, DIM + col:DIM + col + P]).astype(NPBF16)
        wv = _pack(w_qkv[:, 2 * DIM + col:2 * DIM + col + P]).astype(NPBF16)
        bq = (b_qkv[col:col + P] * SCALE).astype(np.float32).reshape(P, 1)
        bv = b_qkv[2 * DIM + col:2 * DIM + col + P].astype(
            np.float32).reshape(P, 1)
        wo = np.ascontiguousarray(w_out[col:col + P, :]).astype(NPBF16)
        in_maps.append(dict(
            xt=xt[b], wq=wq, wk=wk, wv=wv, bq=bq, bv=bv, msk=msk, wo=wo,
        ))
    return in_maps, perm, slices


def run(inputs, trace=False, trace_cores=None):
    in_maps, perm, slices = make_in_maps(
        inputs["x"], inputs["routes"], inputs["w_qkv"], inputs["b_qkv"],
        inputs["w_out"],
    )
    nc = build_nc(slices)
    res = run_bass_kernel_spmd(
        nc, in_maps, list(range(8)), trace=trace, trace_cores=trace_cores,
    )
    b_out = np.asarray(inputs["b_out"], np.float32)
    final = np.zeros((B, S, DIM), np.float32)
    for core in range(8):
        final[core // 4][perm] += np.asarray(
            res.results[core]["out"], np.float32)
    final += b_out[None, None, :]
    return final, res


def kernel(**inputs):
    final, _ = run(inputs, trace=False)
    return final


# revision 15
# speedup vs baseline: 2.0444x; 1.0948x over previous
"""CantorAttention Trainium2 kernel — block-sparse banded attention.

Problem (hardcoded): B=2, S=2048, DIM=512, H=8 heads, D=64, K=64 routes.
  qkv = x @ w_qkv + b_qkv ; per-head sparse attention over routes[q, :] ;
  out = attn_out @ w_out + b_out.

Sharding (8 cores): core i handles batch i//4, heads (2*(i%4), 2*(i%4)+1).
Host gathers: final[b] = sum of the 4 partials of batch b + b_out.

Key idea: routes are k-NN in Cantor-coordinate space. A spectral
seriation of the route graph (host-side) finds a permutation of
positions under which the route matrix is a narrow band: every 128-query
tile's routes fall in a ~229-key window => 2 unaligned 128-key slices.
Attention is computed DENSE per (qtile, slice) block with a
multiplicative count-mask (exact softmax semantics, duplicates
included), skipping everything outside the band: ~5.6x less score/PV/
exp work than full dense.

Softmax denominator: V_aug = [V | ones-col] per slice, so the PV
matmul's row 64 accumulates sum_k pm[k,q] = denominator. den is
replicated across partitions with a tiny selector matmul, reciprocated
and multiplied in per-2-qtile batches.

k-bias is dropped entirely (softmax is invariant to per-query score
shifts); q-bias and the 1/sqrt(D) scale are folded host-side into wq/bq.
"""

import numpy as np
import ml_dtypes

import concourse.bass as bass
import concourse.bacc as bacc
import concourse.mybir as mybir
import concourse.tile as tile
from concourse.bass_utils import run_bass_kernel_spmd
from concourse.masks import make_identity

BF16 = mybir.dt.bfloat16
F32 = mybir.dt.float32
NPBF16 = ml_dtypes.bfloat16

B = 2
S = 2048
DIM = 512
H = 8
D = 64
KR = 64
SCALE = 0.125

P = 128
NQT = S // P      # 16 query tiles
NC4 = DIM // P    # 4 contraction chunks
QC = 512          # phase-1 column chunk
VA = D + 2        # V_aug block stride (64 V + ones col + pad)

_CACHE = {}


def _plan_windows(routes):
    """Host: permutation + per-qtile key-slice offsets from routes alone."""
    routes = np.asarray(routes)
    s = routes.shape[0]
    x = np.arange(s, dtype=np.float64)
    for _ in range(60):
        x = x[routes].mean(1)
        x -= x.mean()
        n = np.linalg.norm(x)
        if n > 0:
            x /= n
    perm = np.argsort(x, kind="stable").astype(np.int64)
    inv = np.empty(s, np.int64)
    inv[perm] = np.arange(s)
    rk = inv[routes[perm]]  # routes in sorted space
    slices = []
    for t in range(s // P):
        r = rk[t * P:(t + 1) * P]
        lo, hi = int(r.min()), int(r.max())
        n_sl = max(2, int(np.ceil((hi - lo + 1) / P)))
        w0 = min(max(0, lo), s - n_sl * P)
        slices.append([w0 + j * P for j in range(n_sl)])
    return perm, inv, rk, slices


def build_nc(slices):
    key = tuple(tuple(s) for s in slices)
    if key in _CACHE:
        return _CACHE[key]
    nsl = [len(s) for s in slices]          # slices per qtile (>=2)
    tot_sl = sum(nsl)                       # total slice count
    sl_base = np.cumsum([0] + nsl).tolist() # block index base per qtile

    nc = bacc.Bacc(
        "TRN2",
        target_bir_lowering=False,
        debug=False,
        num_devices=8,
    )

    xt_d = nc.dram_tensor("xt", [P, NC4 * S], BF16, kind="ExternalInput").ap()
    # all weights packed: wq | wk | wv (4x128 cols each) | wo (512 cols)
    wall_d = nc.dram_tensor("wall", [P, 3 * NC4 * P + DIM], BF16,
                            kind="ExternalInput").ap()
    bqv_d = nc.dram_tensor("bqv", [P, 2], F32, kind="ExternalInput").ap()
    # mask: per qtile, per slice: [128k, 128q] blocks (shared by both heads)
    msk_d = nc.dram_tensor("msk", [P, tot_sl * P], BF16,
                           kind="ExternalInput").ap()
    out_d = nc.dram_tensor("out", [S, DIM], BF16, kind="ExternalOutput").ap()

    with tile.TileContext(nc) as tc:
        with tc.tile_pool(name="persist", bufs=1) as pp:
            ident = pp.tile([P, P], BF16, tag="ident")
            make_identity(nc, ident[:])

            wall_sb = pp.tile([P, 3 * NC4 * P + DIM], BF16, tag="wall")
            nc.sync.dma_start(out=wall_sb[:], in_=wall_d[:, :])
            bqv_sb = pp.tile([P, 2], F32, tag="bqv")
            nc.sync.dma_start(out=bqv_sb[:], in_=bqv_d[:, :])
            w_sb = {n: wall_sb[:, i * NC4 * P:(i + 1) * NC4 * P]
                    for i, n in enumerate(("q", "k", "v"))}
            wo_sb = wall_sb[:, 3 * NC4 * P:]

            xt_sb = pp.tile([P, NC4 * S], BF16, tag="xt")
            msk_sb = pp.tile([P, tot_sl * P], BF16, tag="msk")
            # interleave x chunks with mask halves so masks land before
            # the attention loop starts but x chunk qc is there for phase 1
            mw = tot_sl * P
            mh = (tot_sl // 2) * P

            def dma_x(qc):
                nc.sync.dma_start(
                    out=xt_sb[:].rearrange("p (c s) -> p c s", c=NC4)[
                        :, :, qc * QC:(qc + 1) * QC],
                    in_=xt_d[:, :].rearrange("p (c s) -> p c s", c=NC4)[
                        :, :, qc * QC:(qc + 1) * QC])

            dma_x(0)
            dma_x(1)
            nc.sync.dma_start(out=msk_sb[:, 0:mh], in_=msk_d[:, 0:mh])
            dma_x(2)
            dma_x(3)
            nc.sync.dma_start(out=msk_sb[:, mh:mw], in_=msk_d[:, mh:mw])

            # q^T/k^T per head, rows 64-127 zero-padded so every score
            # matmul is a full 128-contraction base-0 operand.
            qT = [pp.tile([P, S], BF16, tag=f"qT{h}", name=f"qT{h}")
                  for h in range(2)]
            kT = [pp.tile([P, S], BF16, tag=f"kT{h}", name=f"kT{h}")
                  for h in range(2)]
            for h in range(2):
                nc.gpsimd.memset(qT[h][D:P, :], 0.0)
                nc.gpsimd.memset(kT[h][D:P, :], 0.0)
            vT = pp.tile([P, S], BF16, tag="vT")

            # V_aug per (qtile, slice, head): [128k, 64+ones] stride-66
            VAUG = pp.tile([P, 2 * tot_sl * VA], BF16, tag="vaug")
            nc.gpsimd.memset(
                VAUG[:].rearrange("p (b va) -> p b va", va=VA)[:, :, D:D + 1],
                1.0)
            ON = pp.tile([P, S], BF16, tag="on")
            SEL = pp.tile([D + 1, D], BF16, tag="sel")
            nc.vector.memset(SEL[0:D, :], 0.0)
            nc.vector.memset(SEL[D:D + 1, :], 1.0)
            OTS = pp.tile([D + 1, NQT * 2 * P], BF16, tag="ots")

            # ---- Phase 1: QKV^T = W^T @ X^T (k needs no bias) ----
            with tc.tile_pool(name="ph1", bufs=4, space="PSUM") as ph1:
                for qc in range(NC4):
                    cs = slice(qc * QC, (qc + 1) * QC)
                    for name in ("k", "q", "v"):
                        ps = ph1.tile([P, QC], F32, tag="qkv", name="qkv_ps")
                        for c in range(NC4):
                            nc.tensor.matmul(
                                ps[:],
                                lhsT=w_sb[name][:, c * P:(c + 1) * P],
                                rhs=xt_sb[:, c * S + qc * QC:
                                          c * S + (qc + 1) * QC],
                                start=(c == 0),
                                stop=(c == NC4 - 1),
                            )
                        if name == "q":
                            for h in range(2):
                                hd = h * D
                                nc.scalar.activation(
                                    qT[h][0:D, cs], ps[hd:hd + D, :],
                                    mybir.ActivationFunctionType.Identity,
                                    bias=bqv_sb[hd:hd + D, 0:1],
                                )
                        elif name == "k":
                            for h in range(2):
                                hd = h * D
                                nc.vector.tensor_copy(
                                    out=kT[h][0:D, cs], in_=ps[hd:hd + D, :])
                        else:
                            nc.scalar.activation(
                                vT[:, cs], ps[:],
                                mybir.ActivationFunctionType.Identity,
                                bias=bqv_sb[:, 1:2],
                            )

            # ---- V_aug transposes (fills the PE while phase-1 copies
            #      drain; pool closed afterwards to free its banks) ----
            with tc.tile_pool(name="vtp", bufs=2, space="PSUM") as vtp:
                for t in range(NQT):
                    sl = slices[t]
                    ns = len(sl)
                    base = sl_base[t]
                    vt_ps = vtp.tile([P, ns * P], BF16, tag="vt",
                                     name="vt_ps")
                    for j, w in enumerate(sl):
                        nc.tensor.transpose(
                            out=vt_ps[:, j * P:(j + 1) * P],
                            in_=vT[:, w:w + P],
                            identity=ident[:],
                        )
                    vg0 = 2 * base * VA
                    nc.vector.tensor_copy(
                        out=VAUG[:, vg0:vg0 + 2 * ns * VA].rearrange(
                            "p (b va) -> p b va", va=VA)[:, :, 0:D],
                        in_=vt_ps[:].rearrange(
                            "p (b d) -> p b d", d=D))

            # ---- Phase 2: banded attention, 3-stage skewed pipeline ----
            with tc.tile_pool(name="sp", bufs=3, space="PSUM") as sp, \
                 tc.tile_pool(name="otp", bufs=3, space="PSUM") as otp, \
                 tc.tile_pool(name="prp", bufs=2, space="PSUM") as prp, \
                 tc.tile_pool(name="pmp", bufs=3) as pmp, \
                 tc.tile_pool(name="obp", bufs=3) as obp:
                fstate = {}
                mstate = {}

                def stage_front(t):
                    """scores -> exp -> mask for qtile t"""
                    sl = slices[t]
                    ns = len(sl)
                    qs = slice(t * P, (t + 1) * P)
                    base = sl_base[t]
                    sc = sp.tile([P, 2 * ns * P], F32, tag="s", name="s_ps")
                    for h in range(2):
                        for j, w in enumerate(sl):
                            col = (ns * h + j) * P
                            nc.tensor.matmul(
                                sc[:, col:col + P],
                                lhsT=kT[h][:, w:w + P],
                                rhs=qT[h][:, qs],
                                start=True,
                                stop=True,
                            )
                    pm = pmp.tile([P, 2 * ns * P], BF16, tag="pm",
                                  name="pm_sb")
                    nc.scalar.activation(
                        pm[:], sc[:], mybir.ActivationFunctionType.Exp)
                    mcol = base * P
                    mseg = msk_sb[:, mcol:mcol + ns * P]
                    nc.vector.tensor_tensor(
                        out=pm[:].rearrange("p (h c) -> p h c", h=2),
                        in0=pm[:].rearrange("p (h c) -> p h c", h=2),
                        in1=mseg.rearrange("p (o c) -> p o c", o=1)
                            .to_broadcast([P, 2, ns * P]),
                        op=mybir.AluOpType.mult,
                    )
                    fstate[t] = pm

                def stage_mid(t):
                    """PV accumulate; per 2 qtiles: ot copy + den-replicate"""
                    sl = slices[t]
                    ns = len(sl)
                    pm = fstate.pop(t)
                    base = sl_base[t]
                    vg0 = 2 * base * VA
                    if t % 2 == 0:
                        ot = otp.tile([P, 4 * P], F32, tag="ot", name="ot_ps")
                        mstate[t] = ot
                    else:
                        ot = mstate[t - 1]
                    go = (t % 2) * 2 * P
                    for h in range(2):
                        oc = go + h * P
                        for j in range(ns):
                            va = vg0 + (2 * j + h) * VA
                            pc = (ns * h + j) * P
                            nc.tensor.matmul(
                                ot[0:D + 1, oc:oc + P],
                                lhsT=VAUG[:, va:va + D + 1],
                                rhs=pm[:, pc:pc + P],
                                start=(j == 0),
                                stop=(j == ns - 1),
                            )
                    if t % 2 == 1:
                        g = t // 2
                        oc0 = g * 4 * P
                        nc.scalar.copy(
                            out=OTS[:, oc0:oc0 + 4 * P], in_=ot[0:D + 1, :])
                        r2 = otp.tile([P, 4 * P], F32, tag="ot", name="r2")
                        nc.tensor.matmul(
                            r2[0:D, :],
                            lhsT=SEL[:],
                            rhs=OTS[:, oc0:oc0 + 4 * P],
                            start=True, stop=True,
                        )
                        mstate[t] = r2

                def stage_tail(todd):
                    """per 2 qtiles: recip + normalize + project + store"""
                    g = todd // 2
                    oc0 = g * 4 * P
                    del mstate[todd - 1]
                    r2 = mstate.pop(todd)
                    rr = pmp.tile([D, 4 * P], F32, tag="rr", name="rr")
                    nc.vector.reciprocal_approx_fast(
                        out=rr[:], in_=r2[0:D, :])
                    qs2 = slice(2 * g * P, (2 * g + 2) * P)
                    for h in range(2):
                        nc.vector.tensor_tensor(
                            out=ON[h * D:(h + 1) * D, qs2].rearrange(
                                "p (b c) -> p b c", b=2),
                            in0=OTS[0:D, oc0:oc0 + 4 * P].rearrange(
                                "p (b hh c) -> p b hh c", b=2, hh=2)[
                                :, :, h, :],
                            in1=rr[:].rearrange(
                                "p (b hh c) -> p b hh c", b=2, hh=2)[
                                :, :, h, :],
                            op=mybir.AluOpType.mult,
                        )
                    for t in (todd - 1, todd):
                        qs = slice(t * P, (t + 1) * P)
                        pr = prp.tile([P, DIM], F32, tag="pr", name="pr_ps")
                        nc.tensor.matmul(
                            pr[:], lhsT=ON[:, qs], rhs=wo_sb,
                            start=True, stop=True,
                        )
                        ob = obp.tile([P, DIM], BF16, tag="ob", name="ob_sb")
                        if t % 2 == 0:
                            nc.scalar.copy(out=ob[:], in_=pr[:])
                        else:
                            nc.vector.tensor_copy(out=ob[:], in_=pr[:])
                        nc.sync.dma_start(out=out_d[qs, :], in_=ob[:])

                stage_front(0)
                stage_front(1)
                for t in range(NQT):
                    if t + 2 < NQT:
                        stage_front(t + 2)
                    stage_mid(t)
                    if t % 2 == 1:
                        stage_tail(t)

    nc.compile()
    _CACHE[key] = nc
    return nc


def _pack(a):
    # [n*128, X] -> [128, n*X] grouping row-blocks along columns
    n = a.shape[0] // P
    return np.ascontiguousarray(
        a.reshape(n, P, a.shape[1]).transpose(1, 0, 2).reshape(P, -1))


def make_in_maps(x, routes, w_qkv, b_qkv, w_out):
    x = np.asarray(x, np.float32)
    routes = np.asarray(routes)
    w_qkv = np.asarray(w_qkv, np.float32)
    b_qkv = np.asarray(b_qkv, np.float32)
    w_out = np.asarray(w_out, np.float32)

    perm, inv, rk, slices = _plan_windows(routes)

    # count-mask blocks in permuted space: C~[k, q]
    Ct = np.zeros((S, S), np.float32)
    np.add.at(Ct, (rk.ravel(),
                   np.repeat(np.arange(S), KR)), 1.0)
    msk_cols = []
    for t, sl in enumerate(slices):
        for w in sl:
            msk_cols.append(Ct[w:w + P, t * P:(t + 1) * P])
    msk = np.ascontiguousarray(
        np.concatenate(msk_cols, axis=1).astype(NPBF16))

    xt = [_pack(np.ascontiguousarray(x[b][perm].T)).astype(NPBF16)
          for b in range(B)]

    in_maps = []
    for core in range(8):
        b = core // 4
        hp = core % 4
        col = hp * P
        wq = _pack(w_qkv[:, col:col + P] * SCALE)
        wk = _pack(w_qkv[:, DIM + col:DIM + col + P])
        wv = _pack(w_qkv[:, 2 * DIM + col:2 * DIM + col + P])
        wo = np.ascontiguousarray(w_out[col:col + P, :])
        wall = np.concatenate([wq, wk, wv, wo], axis=1).astype(NPBF16)
        bq = (b_qkv[col:col + P] * SCALE).astype(np.float32)
        bv = b_qkv[2 * DIM + col:2 * DIM + col + P].astype(np.float32)
        bqv = np.stack([bq, bv], axis=1)
        in_maps.append(dict(xt=xt[b], wall=wall, bqv=bqv, msk=msk))
    return in_maps, perm, slices


def run(inputs, trace=False, trace_cores=None):
    in_maps, perm, slices = make_in_maps(
        inputs["x"], inputs["routes"], inputs["w_qkv"], inputs["b_qkv"],
        inputs["w_out"],
    )
    nc = build_nc(slices)
    res = run_bass_kernel_spmd(
        nc, in_maps, list(range(8)), trace=trace, trace_cores=trace_cores,
    )
    b_out = np.asarray(inputs["b_out"], np.float32)
    final = np.zeros((B, S, DIM), np.float32)
    for core in range(8):
        final[core // 4][perm] += np.asarray(
            res.results[core]["out"], np.float32)
    final += b_out[None, None, :]
    return final, res


def kernel(**inputs):
    final, _ = run(inputs, trace=False)
    return final


# revision 17
# speedup vs baseline: 2.2047x; 1.0784x over previous
"""CantorAttention Trainium2 kernel — block-sparse banded attention.

Problem (hardcoded): B=2, S=2048, DIM=512, H=8 heads, D=64, K=64 routes.
  qkv = x @ w_qkv + b_qkv ; per-head sparse attention over routes[q, :] ;
  out = attn_out @ w_out + b_out.

Sharding (8 cores): core i handles batch i//4, heads (2*(i%4), 2*(i%4)+1).
Host gathers: final[b] = sum of the 4 partials of batch b + b_out.

Key idea: routes are k-NN in Cantor-coordinate space. A spectral
seriation of the route graph (host-side) finds a permutation of
positions under which the route matrix is a narrow band: every 128-query
tile's routes fall in a ~229-key window => 2 unaligned 128-key slices.
Attention is computed DENSE per (qtile, slice) block with a
multiplicative count-mask (exact softmax semantics, duplicates
included), skipping everything outside the band: ~5.6x less score/PV/
exp work than full dense.

Softmax denominator: V_aug = [V | ones-col] per slice, so the PV
matmul's row 64 accumulates sum_k pm[k,q] = denominator. den is
replicated across partitions with a tiny selector matmul, reciprocated
and multiplied in per-2-qtile batches.

k-bias is dropped entirely (softmax is invariant to per-query score
shifts); q-bias and the 1/sqrt(D) scale are folded host-side into wq/bq.
"""

import numpy as np
import ml_dtypes

import concourse.bass as bass
import concourse.bacc as bacc
import concourse.mybir as mybir
import concourse.tile as tile
from concourse.bass_utils import run_bass_kernel_spmd
from concourse.masks import make_identity

BF16 = mybir.dt.bfloat16
F32 = mybir.dt.float32
NPBF16 = ml_dtypes.bfloat16
FP8 = mybir.dt.float8e4
NPFP8 = ml_dtypes.float8_e4m3

B = 2
S = 2048
DIM = 512
H = 8
D = 64
KR = 64
SCALE = 0.125

P = 128
NQT = S // P      # 16 query tiles
NC4 = DIM // P    # 4 contraction chunks
QC = 512          # phase-1 column chunk
VA = D + 2        # V_aug block stride (64 V + ones col + pad)

_CACHE = {}


def _plan_windows(routes):
    """Host: permutation + per-qtile key-slice offsets from routes alone."""
    routes = np.asarray(routes)
    s = routes.shape[0]
    x = np.arange(s, dtype=np.float64)
    for _ in range(60):
        x = x[routes].mean(1)
        x -= x.mean()
        n = np.linalg.norm(x)
        if n > 0:
            x /= n
    perm = np.argsort(x, kind="stable").astype(np.int64)
    inv = np.empty(s, np.int64)
    inv[perm] = np.arange(s)
    rk = inv[routes[perm]]  # routes in sorted space
    slices = []
    for t in range(s // P):
        r = rk[t * P:(t + 1) * P]
        lo, hi = int(r.min()), int(r.max())
        n_sl = max(2, int(np.ceil((hi - lo + 1) / P)))
        w0 = min(max(0, lo), s - n_sl * P)
        slices.append([w0 + j * P for j in range(n_sl)])
    return perm, inv, rk, slices


def build_nc(slices):
    key = tuple(tuple(s) for s in slices)
    if key in _CACHE:
        return _CACHE[key]
    nsl = [len(s) for s in slices]          # slices per qtile (>=2)
    tot_sl = sum(nsl)                       # total slice count
    sl_base = np.cumsum([0] + nsl).tolist() # block index base per qtile

    nc = bacc.Bacc(
        "TRN2",
        target_bir_lowering=False,
        debug=False,
        num_devices=8,
    )

    xt_d = nc.dram_tensor("xt", [P, NC4 * S], BF16, kind="ExternalInput").ap()
    # qkv weights packed: wq | wk | wv (4x128 cols each)
    w8_d = nc.dram_tensor("w8", [P, 3 * NC4 * P], BF16,
                          kind="ExternalInput").ap()
    wo_d = nc.dram_tensor("wo", [P, DIM], BF16, kind="ExternalInput").ap()
    bqv_d = nc.dram_tensor("bqv", [P, 2], F32, kind="ExternalInput").ap()
    # mask: per qtile, per slice: [128k, 128q] blocks (shared by both heads)
    msk_d = nc.dram_tensor("msk", [P, tot_sl * P], BF16,
                           kind="ExternalInput").ap()
    out_d = nc.dram_tensor("out", [S, DIM], BF16, kind="ExternalOutput").ap()

    with tile.TileContext(nc) as tc:
        with tc.tile_pool(name="persist", bufs=1) as pp:
            ident = pp.tile([P, P], BF16, tag="ident")
            make_identity(nc, ident[:])

            w8_sb = pp.tile([P, 3 * NC4 * P], BF16, tag="w8")
            nc.sync.dma_start(out=w8_sb[:], in_=w8_d[:, :])
            wo_t = pp.tile([P, DIM], BF16, tag="wo")
            nc.sync.dma_start(out=wo_t[:], in_=wo_d[:, :])
            wo_sb = wo_t[:]
            bqv_sb = pp.tile([P, 2], F32, tag="bqv")
            nc.sync.dma_start(out=bqv_sb[:], in_=bqv_d[:, :])
            w_sb = {n: w8_sb[:, i * NC4 * P:(i + 1) * NC4 * P]
                    for i, n in enumerate(("q", "k", "v"))}

            xt_sb = pp.tile([P, NC4 * S], BF16, tag="xt")
            msk_sb = pp.tile([P, tot_sl * P], BF16, tag="msk")
            # interleave x chunks with mask halves so masks land before
            # the attention loop starts but x chunk qc is there for phase 1
            mw = tot_sl * P
            mh = (tot_sl // 2) * P

            def dma_x(qc):
                nc.sync.dma_start(
                    out=xt_sb[:].rearrange("p (c s) -> p c s", c=NC4)[
                        :, :, qc * QC:(qc + 1) * QC],
                    in_=xt_d[:, :].rearrange("p (c s) -> p c s", c=NC4)[
                        :, :, qc * QC:(qc + 1) * QC])

            dma_x(0)
            dma_x(1)
            nc.sync.dma_start(out=msk_sb[:, 0:mh], in_=msk_d[:, 0:mh])
            dma_x(2)
            dma_x(3)
            nc.sync.dma_start(out=msk_sb[:, mh:mw], in_=msk_d[:, mh:mw])

            # q^T/k^T per head, rows 64-127 zero-padded so every score
            # matmul is a full 128-contraction base-0 operand.
            qT = [pp.tile([P, S], BF16, tag=f"qT{h}", name=f"qT{h}")
                  for h in range(2)]
            kT = [pp.tile([P, S], BF16, tag=f"kT{h}", name=f"kT{h}")
                  for h in range(2)]
            for h in range(2):
                nc.gpsimd.memset(qT[h][D:P, :], 0.0)
                nc.gpsimd.memset(kT[h][D:P, :], 0.0)
            vT = pp.tile([P, S], BF16, tag="vT")

            # V_aug per (qtile, slice, head): [128k, 64+ones] stride-66
            VAUG = pp.tile([P, 2 * tot_sl * VA], BF16, tag="vaug")
            nc.gpsimd.memset(
                VAUG[:].rearrange("p (b va) -> p b va", va=VA)[:, :, D:D + 1],
                1.0)
            ON = pp.tile([P, S], BF16, tag="on")
            SEL = pp.tile([D + 1, D], BF16, tag="sel")
            nc.vector.memset(SEL[0:D, :], 0.0)
            nc.vector.memset(SEL[D:D + 1, :], 1.0)
            OTS = pp.tile([D + 1, NQT * 2 * P], BF16, tag="ots")

            # ---- Phase 1: QKV^T = W^T @ X^T, fp8 DoubleRow (2 contraction
            #      chunks per matmul); V_aug transposes interleaved as soon
            #      as the chunks covering each qtile's slices are ready ----
            xt3 = xt_sb[:].rearrange("p (c s) -> p c s", c=NC4)
            vt_done = 0
            with tc.tile_pool(name="ph1", bufs=4, space="PSUM") as ph1:

                def emit_vt(t):
                    sl = slices[t]
                    ns = len(sl)
                    base = sl_base[t]
                    vt_ps = ph1.tile([P, ns * P], BF16, tag="vt",
                                     name="vt_ps")
                    for j, w in enumerate(sl):
                        nc.tensor.transpose(
                            out=vt_ps[:, j * P:(j + 1) * P],
                            in_=vT[:, w:w + P],
                            identity=ident[:],
                        )
                    vg0 = 2 * base * VA
                    nc.vector.tensor_copy(
                        out=VAUG[:, vg0:vg0 + 2 * ns * VA].rearrange(
                            "p (b va) -> p b va", va=VA)[:, :, 0:D],
                        in_=vt_ps[:].rearrange(
                            "p (b d) -> p b d", d=D))

                for qc in range(NC4):
                    cs = slice(qc * QC, (qc + 1) * QC)
                    for name in ("k", "q", "v"):
                        ps = ph1.tile([P, QC], F32, tag="qkv", name="qkv_ps")
                        for c in range(NC4):
                            nc.tensor.matmul(
                                ps[:],
                                lhsT=w_sb[name][:, c * P:(c + 1) * P],
                                rhs=xt3[:, c, cs],
                                start=(c == 0),
                                stop=(c == NC4 - 1),
                            )
                        if name == "q":
                            for h in range(2):
                                hd = h * D
                                nc.scalar.activation(
                                    qT[h][0:D, cs], ps[hd:hd + D, :],
                                    mybir.ActivationFunctionType.Identity,
                                    bias=bqv_sb[hd:hd + D, 0:1],
                                )
                        elif name == "k":
                            for h in range(2):
                                hd = h * D
                                nc.vector.tensor_copy(
                                    out=kT[h][0:D, cs], in_=ps[hd:hd + D, :])
                        else:
                            nc.scalar.activation(
                                vT[:, cs], ps[:],
                                mybir.ActivationFunctionType.Identity,
                                bias=bqv_sb[:, 1:2],
                            )
                    lim = (qc + 1) * QC
                    while (vt_done < NQT and
                           slices[vt_done][-1] + P <= lim):
                        emit_vt(vt_done)
                        vt_done += 1
                while vt_done < NQT:
                    emit_vt(vt_done)
                    vt_done += 1

            # ---- Phase 2: banded attention, 3-stage skewed pipeline ----
            with tc.tile_pool(name="sp", bufs=3, space="PSUM") as sp, \
                 tc.tile_pool(name="otp", bufs=3, space="PSUM") as otp, \
                 tc.tile_pool(name="prp", bufs=2, space="PSUM") as prp, \
                 tc.tile_pool(name="pmp", bufs=3) as pmp, \
                 tc.tile_pool(name="obp", bufs=3) as obp:
                fstate = {}
                mstate = {}

                def stage_front(t):
                    """scores -> exp -> mask for qtile t"""
                    sl = slices[t]
                    ns = len(sl)
                    qs = slice(t * P, (t + 1) * P)
                    base = sl_base[t]
                    sc = sp.tile([P, 2 * ns * P], F32, tag="s", name="s_ps")
                    for h in range(2):
                        for j, w in enumerate(sl):
                            col = (ns * h + j) * P
                            nc.tensor.matmul(
                                sc[:, col:col + P],
                                lhsT=kT[h][:, w:w + P],
                                rhs=qT[h][:, qs],
                                start=True,
                                stop=True,
                            )
                    pm = pmp.tile([P, 2 * ns * P], BF16, tag="pm",
                                  name="pm_sb")
                    nc.scalar.activation(
                        pm[:], sc[:], mybir.ActivationFunctionType.Exp,
                        scale=SCALE)
                    mcol = base * P
                    mseg = msk_sb[:, mcol:mcol + ns * P]
                    nc.vector.tensor_tensor(
                        out=pm[:].rearrange("p (h c) -> p h c", h=2),
                        in0=pm[:].rearrange("p (h c) -> p h c", h=2),
                        in1=mseg.rearrange("p (o c) -> p o c", o=1)
                            .to_broadcast([P, 2, ns * P]),
                        op=mybir.AluOpType.mult,
                    )
                    fstate[t] = pm

                def stage_mid(t):
                    """PV accumulate; per 2 qtiles: ot copy + den-replicate"""
                    sl = slices[t]
                    ns = len(sl)
                    pm = fstate.pop(t)
                    base = sl_base[t]
                    vg0 = 2 * base * VA
                    if t % 2 == 0:
                        ot = otp.tile([P, 4 * P], F32, tag="ot", name="ot_ps")
                        mstate[t] = ot
                    else:
                        ot = mstate[t - 1]
                    go = (t % 2) * 2 * P
                    for h in range(2):
                        oc = go + h * P
                        for j in range(ns):
                            va = vg0 + (2 * j + h) * VA
                            pc = (ns * h + j) * P
                            nc.tensor.matmul(
                                ot[0:D + 1, oc:oc + P],
                                lhsT=VAUG[:, va:va + D + 1],
                                rhs=pm[:, pc:pc + P],
                                start=(j == 0),
                                stop=(j == ns - 1),
                            )
                    if t % 2 == 1:
                        g = t // 2
                        oc0 = g * 4 * P
                        nc.scalar.copy(
                            out=OTS[:, oc0:oc0 + 4 * P], in_=ot[0:D + 1, :])
                        r2 = otp.tile([P, 4 * P], F32, tag="ot", name="r2")
                        nc.tensor.matmul(
                            r2[0:D, :],
                            lhsT=SEL[:],
                            rhs=OTS[:, oc0:oc0 + 4 * P],
                            start=True, stop=True,
                        )
                        mstate[t] = r2

                def stage_tail(todd):
                    """per 2 qtiles: recip + normalize + project + store"""
                    g = todd // 2
                    oc0 = g * 4 * P
                    del mstate[todd - 1]
                    r2 = mstate.pop(todd)
                    rr = pmp.tile([D, 4 * P], F32, tag="rr", name="rr")
                    nc.vector.reciprocal_approx_fast(
                        out=rr[:], in_=r2[0:D, :])
                    qs2 = slice(2 * g * P, (2 * g + 2) * P)
                    for h in range(2):
                        nc.vector.tensor_tensor(
                            out=ON[h * D:(h + 1) * D, qs2].rearrange(
                                "p (b c) -> p b c", b=2),
                            in0=OTS[0:D, oc0:oc0 + 4 * P].rearrange(
                                "p (b hh c) -> p b hh c", b=2, hh=2)[
                                :, :, h, :],
                            in1=rr[:].rearrange(
                                "p (b hh c) -> p b hh c", b=2, hh=2)[
                                :, :, h, :],
                            op=mybir.AluOpType.mult,
                        )
                    ob = obp.tile([P, 2 * DIM], BF16, tag="ob",
                                  name="ob_sb")
                    for i, t in enumerate((todd - 1, todd)):
                        qs = slice(t * P, (t + 1) * P)
                        pr = prp.tile([P, DIM], F32, tag="pr", name="pr_ps")
                        nc.tensor.matmul(
                            pr[:], lhsT=ON[:, qs], rhs=wo_sb,
                            start=True, stop=True,
                        )
                        if t % 2 == 0:
                            nc.scalar.copy(
                                out=ob[:, i * DIM:(i + 1) * DIM], in_=pr[:])
                        else:
                            nc.vector.tensor_copy(
                                out=ob[:, i * DIM:(i + 1) * DIM], in_=pr[:])
                    nc.sync.dma_start(
                        out=out_d[(todd - 1) * P:(todd + 1) * P, :]
                            .rearrange("(b p) c -> p b c", b=2),
                        in_=ob[:].rearrange("p (b c) -> p b c", b=2))

                stage_front(0)
                stage_front(1)
                for t in range(NQT):
                    if t + 2 < NQT:
                        stage_front(t + 2)
                    stage_mid(t)
                    if t % 2 == 1:
                        stage_tail(t)

    nc.compile()
    _CACHE[key] = nc
    return nc


def _pack(a):
    # [n*128, X] -> [128, n*X] grouping row-blocks along columns
    n = a.shape[0] // P
    return np.ascontiguousarray(
        a.reshape(n, P, a.shape[1]).transpose(1, 0, 2).reshape(P, -1))


def make_in_maps(x, routes, w_qkv, b_qkv, w_out):
    x = np.asarray(x, np.float32)
    routes = np.asarray(routes)
    w_qkv = np.asarray(w_qkv, np.float32)
    b_qkv = np.asarray(b_qkv, np.float32)
    w_out = np.asarray(w_out, np.float32)

    perm, inv, rk, slices = _plan_windows(routes)

    # count-mask blocks in permuted space: C~[k, q]
    Ct = np.zeros((S, S), np.float32)
    np.add.at(Ct, (rk.ravel(),
                   np.repeat(np.arange(S), KR)), 1.0)
    msk_cols = []
    for t, sl in enumerate(slices):
        for w in sl:
            msk_cols.append(Ct[w:w + P, t * P:(t + 1) * P])
    msk = np.ascontiguousarray(
        np.concatenate(msk_cols, axis=1).astype(NPBF16))

    xt = [_pack(np.ascontiguousarray(x[b][perm].T)).astype(NPBF16)
          for b in range(B)]

    in_maps = []
    for core in range(8):
        b = core // 4
        hp = core % 4
        col = hp * P
        wq = _pack(w_qkv[:, col:col + P])
        wk = _pack(w_qkv[:, DIM + col:DIM + col + P])
        wv = _pack(w_qkv[:, 2 * DIM + col:2 * DIM + col + P])
        w8 = np.concatenate([wq, wk, wv], axis=1).astype(NPBF16)
        wo = np.ascontiguousarray(w_out[col:col + P, :]).astype(NPBF16)
        bq = b_qkv[col:col + P].astype(np.float32)
        bv = b_qkv[2 * DIM + col:2 * DIM + col + P].astype(np.float32)
        bqv = np.stack([bq, bv], axis=1)
        in_maps.append(dict(xt=xt[b], w8=w8, wo=wo, bqv=bqv, msk=msk))
    return in_maps, perm, slices


def run(inputs, trace=False, trace_cores=None):
    in_maps, perm, slices = make_in_maps(
        inputs["x"], inputs["routes"], inputs["w_qkv"], inputs["b_qkv"],
        inputs["w_out"],
    )
    nc = build_nc(slices)
    res = run_bass_kernel_spmd(
        nc, in_maps, list(range(8)), trace=trace, trace_cores=trace_cores,
    )
    b_out = np.asarray(inputs["b_out"], np.float32)
    final = np.zeros((B, S, DIM), np.float32)
    for core in range(8):
        final[core // 4][perm] += np.asarray(
            res.results[core]["out"], np.float32)
    final += b_out[None, None, :]
    return final, res


def kernel(**inputs):
    final, _ = run(inputs, trace=False)
    return final


# revision 18
# speedup vs baseline: 2.2167x; 1.0054x over previous
"""CantorAttention Trainium2 kernel — block-sparse banded attention.

Problem (hardcoded): B=2, S=2048, DIM=512, H=8 heads, D=64, K=64 routes.
  qkv = x @ w_qkv + b_qkv ; per-head sparse attention over routes[q, :] ;
  out = attn_out @ w_out + b_out.

Sharding (8 cores): core i handles batch i//4, heads (2*(i%4), 2*(i%4)+1).
Host gathers: final[b] = sum of the 4 partials of batch b + b_out.

Key idea: routes are k-NN in Cantor-coordinate space. A spectral
seriation of the route graph (host-side) finds a permutation of
positions under which the route matrix is a narrow band: every 128-query
tile's routes fall in a ~229-key window => 2 unaligned 128-key slices.
Attention is computed DENSE per (qtile, slice) block with a
multiplicative count-mask (exact softmax semantics, duplicates
included), skipping everything outside the band: ~5.6x less score/PV/
exp work than full dense.

Softmax denominator: V_aug = [V | ones-col] per slice, so the PV
matmul's row 64 accumulates sum_k pm[k,q] = denominator. den is
replicated across partitions with a tiny selector matmul, reciprocated
and multiplied in per-2-qtile batches.

k-bias is dropped entirely (softmax is invariant to per-query score
shifts); q-bias and the 1/sqrt(D) scale are folded host-side into wq/bq.
"""

import numpy as np
import ml_dtypes

import concourse.bass as bass
import concourse.bacc as bacc
import concourse.mybir as mybir
import concourse.tile as tile
from concourse.bass_utils import run_bass_kernel_spmd
from concourse.masks import make_identity

BF16 = mybir.dt.bfloat16
F32 = mybir.dt.float32
NPBF16 = ml_dtypes.bfloat16
FP8 = mybir.dt.float8e4
NPFP8 = ml_dtypes.float8_e4m3

B = 2
S = 2048
DIM = 512
H = 8
D = 64
KR = 64
SCALE = 0.125

P = 128
NQT = S // P      # 16 query tiles
NC4 = DIM // P    # 4 contraction chunks
QC = 512          # phase-1 column chunk
VA = D + 2        # V_aug block stride (64 V + ones col + pad)

_CACHE = {}


def _plan_windows(routes):
    """Host: permutation + per-qtile key-slice offsets from routes alone."""
    routes = np.asarray(routes)
    s = routes.shape[0]
    x = np.arange(s, dtype=np.float64)
    for _ in range(60):
        x = x[routes].mean(1)
        x -= x.mean()
        n = np.linalg.norm(x)
        if n > 0:
            x /= n
    perm = np.argsort(x, kind="stable").astype(np.int64)
    inv = np.empty(s, np.int64)
    inv[perm] = np.arange(s)
    rk = inv[routes[perm]]  # routes in sorted space
    slices = []
    for t in range(s // P):
        r = rk[t * P:(t + 1) * P]
        lo, hi = int(r.min()), int(r.max())
        n_sl = max(2, int(np.ceil((hi - lo + 1) / P)))
        w0 = min(max(0, lo), s - n_sl * P)
        slices.append([w0 + j * P for j in range(n_sl)])
    return perm, inv, rk, slices


def build_nc(slices):
    key = tuple(tuple(s) for s in slices)
    if key in _CACHE:
        return _CACHE[key]
    nsl = [len(s) for s in slices]          # slices per qtile (>=2)
    tot_sl = sum(nsl)                       # total slice count
    sl_base = np.cumsum([0] + nsl).tolist() # block index base per qtile

    nc = bacc.Bacc(
        "TRN2",
        target_bir_lowering=False,
        debug=False,
        num_devices=8,
    )

    xt_d = nc.dram_tensor("xt", [P, NC4 * S], BF16, kind="ExternalInput").ap()
    # qkv weights packed: wq | wk | wv (4x128 cols each)
    w8_d = nc.dram_tensor("w8", [P, 3 * NC4 * P], BF16,
                          kind="ExternalInput").ap()
    wo_d = nc.dram_tensor("wo", [P, DIM], BF16, kind="ExternalInput").ap()
    bqv_d = nc.dram_tensor("bqv", [P, 2], F32, kind="ExternalInput").ap()
    # mask: per qtile, per slice: [128k, 128q] blocks (shared by both heads)
    msk_d = nc.dram_tensor("msk", [P, tot_sl * P], BF16,
                           kind="ExternalInput").ap()
    out_d = nc.dram_tensor("out", [S, DIM], BF16, kind="ExternalOutput").ap()

    with tile.TileContext(nc) as tc:
        with tc.tile_pool(name="persist", bufs=1) as pp:
            ident = pp.tile([P, P], BF16, tag="ident")
            make_identity(nc, ident[:])

            w8_sb = pp.tile([P, 3 * NC4 * P], BF16, tag="w8")
            w_sb = {n: w8_sb[:, i * NC4 * P:(i + 1) * NC4 * P]
                    for i, n in enumerate(("q", "k", "v"))}
            wo_t = pp.tile([P, DIM], BF16, tag="wo")
            wo_sb = wo_t[:]
            bqv_sb = pp.tile([P, 2], F32, tag="bqv")

            xt_sb = pp.tile([P, NC4 * S], BF16, tag="xt")
            msk_sb = pp.tile([P, tot_sl * P], BF16, tag="msk")
            mw = tot_sl * P
            mh = (tot_sl // 2) * P
            NW = NC4 * P

            def dma_x(qc):
                nc.sync.dma_start(
                    out=xt_sb[:].rearrange("p (c s) -> p c s", c=NC4)[
                        :, :, qc * QC:(qc + 1) * QC],
                    in_=xt_d[:, :].rearrange("p (c s) -> p c s", c=NC4)[
                        :, :, qc * QC:(qc + 1) * QC])

            # ordered so the first phase-1 matmul (q, chunk 0) unblocks
            # as early as possible; masks land before the attention loop
            nc.sync.dma_start(out=w8_sb[:, 0:NW], in_=w8_d[:, 0:NW])
            nc.sync.dma_start(out=bqv_sb[:], in_=bqv_d[:, :])
            dma_x(0)
            nc.sync.dma_start(out=w8_sb[:, NW:3 * NW], in_=w8_d[:, NW:3 * NW])
            dma_x(1)
            nc.sync.dma_start(out=msk_sb[:, 0:mh], in_=msk_d[:, 0:mh])
            dma_x(2)
            dma_x(3)
            nc.sync.dma_start(out=wo_t[:], in_=wo_d[:, :])
            nc.sync.dma_start(out=msk_sb[:, mh:mw], in_=msk_d[:, mh:mw])

            # q^T/k^T per head, rows 64-127 zero-padded so every score
            # matmul is a full 128-contraction base-0 operand.
            qT = [pp.tile([P, S], BF16, tag=f"qT{h}", name=f"qT{h}")
                  for h in range(2)]
            kT = [pp.tile([P, S], BF16, tag=f"kT{h}", name=f"kT{h}")
                  for h in range(2)]
            for h in range(2):
                nc.gpsimd.memset(qT[h][D:P, :], 0.0)
                nc.gpsimd.memset(kT[h][D:P, :], 0.0)
            vT = pp.tile([P, S], BF16, tag="vT")

            # V_aug per (qtile, slice, head): [128k, 64+ones] stride-66
            VAUG = pp.tile([P, 2 * tot_sl * VA], BF16, tag="vaug")
            nc.gpsimd.memset(
                VAUG[:].rearrange("p (b va) -> p b va", va=VA)[:, :, D:D + 1],
                1.0)
            ON = pp.tile([P, S], BF16, tag="on")
            SEL = pp.tile([D + 1, D], BF16, tag="sel")
            nc.vector.memset(SEL[0:D, :], 0.0)
            nc.vector.memset(SEL[D:D + 1, :], 1.0)
            OTS = pp.tile([D + 1, NQT * 2 * P], BF16, tag="ots")

            # ---- Phase 1: QKV^T = W^T @ X^T, fp8 DoubleRow (2 contraction
            #      chunks per matmul); V_aug transposes interleaved as soon
            #      as the chunks covering each qtile's slices are ready ----
            xt3 = xt_sb[:].rearrange("p (c s) -> p c s", c=NC4)
            vt_done = 0
            with tc.tile_pool(name="ph1", bufs=4, space="PSUM") as ph1:

                def emit_vt(t0, t1):
                    # slices of qtiles t0..t1 are contiguous blocks in VAUG
                    blks = []
                    for t in range(t0, t1 + 1):
                        blks += slices[t]
                    nb = len(blks)
                    vt_ps = ph1.tile([P, nb * P], BF16, tag="vt",
                                     name="vt_ps")
                    for j, w in enumerate(blks):
                        nc.tensor.transpose(
                            out=vt_ps[:, j * P:(j + 1) * P],
                            in_=vT[:, w:w + P],
                            identity=ident[:],
                        )
                    vg0 = 2 * sl_base[t0] * VA
                    nc.vector.tensor_copy(
                        out=VAUG[:, vg0:vg0 + 2 * nb * VA].rearrange(
                            "p (b va) -> p b va", va=VA)[:, :, 0:D],
                        in_=vt_ps[:].rearrange(
                            "p (b d) -> p b d", d=D))

                for qc in range(NC4):
                    cs = slice(qc * QC, (qc + 1) * QC)
                    for name in ("q", "v", "k"):
                        ps = ph1.tile([P, QC], F32, tag="qkv", name="qkv_ps")
                        for c in range(NC4):
                            nc.tensor.matmul(
                                ps[:],
                                lhsT=w_sb[name][:, c * P:(c + 1) * P],
                                rhs=xt3[:, c, cs],
                                start=(c == 0),
                                stop=(c == NC4 - 1),
                            )
                        if name == "q":
                            for h in range(2):
                                hd = h * D
                                nc.scalar.activation(
                                    qT[h][0:D, cs], ps[hd:hd + D, :],
                                    mybir.ActivationFunctionType.Identity,
                                    bias=bqv_sb[hd:hd + D, 0:1],
                                )
                        elif name == "k":
                            for h in range(2):
                                hd = h * D
                                nc.vector.tensor_copy(
                                    out=kT[h][0:D, cs], in_=ps[hd:hd + D, :])
                        else:
                            nc.scalar.activation(
                                vT[:, cs], ps[:],
                                mybir.ActivationFunctionType.Identity,
                                bias=bqv_sb[:, 1:2],
                            )
                    lim = (qc + 1) * QC
                    while (vt_done + 1 < NQT and
                           slices[vt_done + 1][-1] + P <= lim):
                        emit_vt(vt_done, vt_done + 1)
                        vt_done += 2
                while vt_done < NQT:
                    t1 = min(vt_done + 1, NQT - 1)
                    emit_vt(vt_done, t1)
                    vt_done = t1 + 1

            # ---- Phase 2: banded attention, 3-stage skewed pipeline ----
            with tc.tile_pool(name="sp", bufs=3, space="PSUM") as sp, \
                 tc.tile_pool(name="otp", bufs=3, space="PSUM") as otp, \
                 tc.tile_pool(name="prp", bufs=2, space="PSUM") as prp, \
                 tc.tile_pool(name="pmp", bufs=3) as pmp, \
                 tc.tile_pool(name="obp", bufs=3) as obp:
                fstate = {}
                mstate = {}

                def stage_front(t):
                    """scores -> exp -> mask for qtile t"""
                    sl = slices[t]
                    ns = len(sl)
                    qs = slice(t * P, (t + 1) * P)
                    base = sl_base[t]
                    sc = sp.tile([P, 2 * ns * P], F32, tag="s", name="s_ps")
                    for h in range(2):
                        for j, w in enumerate(sl):
                            col = (ns * h + j) * P
                            nc.tensor.matmul(
                                sc[:, col:col + P],
                                lhsT=kT[h][:, w:w + P],
                                rhs=qT[h][:, qs],
                                start=True,
                                stop=True,
                            )
                    pm = pmp.tile([P, 2 * ns * P], BF16, tag="pm",
                                  name="pm_sb")
                    nc.scalar.activation(
                        pm[:], sc[:], mybir.ActivationFunctionType.Exp,
                        scale=SCALE)
                    mcol = base * P
                    mseg = msk_sb[:, mcol:mcol + ns * P]
                    nc.vector.tensor_tensor(
                        out=pm[:].rearrange("p (h c) -> p h c", h=2),
                        in0=pm[:].rearrange("p (h c) -> p h c", h=2),
                        in1=mseg.rearrange("p (o c) -> p o c", o=1)
                            .to_broadcast([P, 2, ns * P]),
                        op=mybir.AluOpType.mult,
                    )
                    fstate[t] = pm

                def stage_mid(t):
                    """PV accumulate; per 2 qtiles: ot copy + den-replicate"""
                    sl = slices[t]
                    ns = len(sl)
                    pm = fstate.pop(t)
                    base = sl_base[t]
                    vg0 = 2 * base * VA
                    if t % 2 == 0:
                        ot = otp.tile([P, 4 * P], F32, tag="ot", name="ot_ps")
                        mstate[t] = ot
                    else:
                        ot = mstate[t - 1]
                    go = (t % 2) * 2 * P
                    for h in range(2):
                        oc = go + h * P
                        for j in range(ns):
                            va = vg0 + (2 * j + h) * VA
                            pc = (ns * h + j) * P
                            nc.tensor.matmul(
                                ot[0:D + 1, oc:oc + P],
                                lhsT=VAUG[:, va:va + D + 1],
                                rhs=pm[:, pc:pc + P],
                                start=(j == 0),
                                stop=(j == ns - 1),
                            )
                    if t % 2 == 1:
                        g = t // 2
                        oc0 = g * 4 * P
                        nc.scalar.copy(
                            out=OTS[:, oc0:oc0 + 4 * P], in_=ot[0:D + 1, :])
                        r2 = otp.tile([P, 4 * P], F32, tag="ot", name="r2")
                        nc.tensor.matmul(
                            r2[0:D, :],
                            lhsT=SEL[:],
                            rhs=OTS[:, oc0:oc0 + 4 * P],
                            start=True, stop=True,
                        )
                        mstate[t] = r2

                def stage_tail(todd):
                    """per 2 qtiles: recip + normalize + project + store"""
                    g = todd // 2
                    oc0 = g * 4 * P
                    del mstate[todd - 1]
                    r2 = mstate.pop(todd)
                    rr = pmp.tile([D, 4 * P], F32, tag="rr", name="rr")
                    nc.vector.reciprocal_approx_fast(
                        out=rr[:], in_=r2[0:D, :])
                    qs2 = slice(2 * g * P, (2 * g + 2) * P)
                    for h in range(2):
                        nc.vector.tensor_tensor(
                            out=ON[h * D:(h + 1) * D, qs2].rearrange(
                                "p (b c) -> p b c", b=2),
                            in0=OTS[0:D, oc0:oc0 + 4 * P].rearrange(
                                "p (b hh c) -> p b hh c", b=2, hh=2)[
                                :, :, h, :],
                            in1=rr[:].rearrange(
                                "p (b hh c) -> p b hh c", b=2, hh=2)[
                                :, :, h, :],
                            op=mybir.AluOpType.mult,
                        )
                    ob = obp.tile([P, 2 * DIM], BF16, tag="ob",
                                  name="ob_sb")
                    for i, t in enumerate((todd - 1, todd)):
                        qs = slice(t * P, (t + 1) * P)
                        pr = prp.tile([P, DIM], F32, tag="pr", name="pr_ps")
                        nc.tensor.matmul(
                            pr[:], lhsT=ON[:, qs], rhs=wo_sb,
                            start=True, stop=True,
                        )
                        if t % 2 == 0:
                            nc.scalar.copy(
                                out=ob[:, i * DIM:(i + 1) * DIM], in_=pr[:])
                        else:
                            nc.vector.tensor_copy(
                                out=ob[:, i * DIM:(i + 1) * DIM], in_=pr[:])
                    nc.sync.dma_start(
                        out=out_d[(todd - 1) * P:(todd + 1) * P, :]
                            .rearrange("(b p) c -> p b c", b=2),
                        in_=ob[:].rearrange("p (b c) -> p b c", b=2))

                stage_front(0)
                stage_front(1)
                for t in range(NQT):
                    if t + 2 < NQT:
                        stage_front(t + 2)
                    stage_mid(t)
                    if t % 2 == 1:
                        stage_tail(t)

    nc.compile()
    _CACHE[key] = nc
    return nc


def _pack(a):
    # [n*128, X] -> [128, n*X] grouping row-blocks along columns
    n = a.shape[0] // P
    return np.ascontiguousarray(
        a.reshape(n, P, a.shape[1]).transpose(1, 0, 2).reshape(P, -1))


def make_in_maps(x, routes, w_qkv, b_qkv, w_out):
    x = np.asarray(x, np.float32)
    routes = np.asarray(routes)
    w_qkv = np.asarray(w_qkv, np.float32)
    b_qkv = np.asarray(b_qkv, np.float32)
    w_out = np.asarray(w_out, np.float32)

    perm, inv, rk, slices = _plan_windows(routes)

    # count-mask blocks in permuted space: C~[k, q]
    Ct = np.zeros((S, S), np.float32)
    np.add.at(Ct, (rk.ravel(),
                   np.repeat(np.arange(S), KR)), 1.0)
    msk_cols = []
    for t, sl in enumerate(slices):
        for w in sl:
            msk_cols.append(Ct[w:w + P, t * P:(t + 1) * P])
    msk = np.ascontiguousarray(
        np.concatenate(msk_cols, axis=1).astype(NPBF16))

    xt = [_pack(np.ascontiguousarray(x[b][perm].T)).astype(NPBF16)
          for b in range(B)]

    in_maps = []
    for core in range(8):
        b = core // 4
        hp = core % 4
        col = hp * P
        wq = _pack(w_qkv[:, col:col + P])
        wk = _pack(w_qkv[:, DIM + col:DIM + col + P])
        wv = _pack(w_qkv[:, 2 * DIM + col:2 * DIM + col + P])
        w8 = np.concatenate([wq, wk, wv], axis=1).astype(NPBF16)
        wo = np.ascontiguousarray(w_out[col:col + P, :]).astype(NPBF16)
        bq = b_qkv[col:col + P].astype(np.float32)
        bv = b_qkv[2 * DIM + col:2 * DIM + col + P].astype(np.float32)
        bqv = np.stack([bq, bv], axis=1)
        in_maps.append(dict(xt=xt[b], w8=w8, wo=wo, bqv=bqv, msk=msk))
    return in_maps, perm, slices


def run(inputs, trace=False, trace_cores=None):
    in_maps, perm, slices = make_in_maps(
        inputs["x"], inputs["routes"], inputs["w_qkv"], inputs["b_qkv"],
        inputs["w_out"],
    )
    nc = build_nc(slices)
    res = run_bass_kernel_spmd(
        nc, in_maps, list(range(8)), trace=trace, trace_cores=trace_cores,
    )
    b_out = np.asarray(inputs["b_out"], np.float32)
    final = np.zeros((B, S, DIM), np.float32)
    for core in range(8):
        final[core // 4][perm] += np.asarray(
            res.results[core]["out"], np.float32)
    final += b_out[None, None, :]
    return final, res


def kernel(**inputs):
    final, _ = run(inputs, trace=False)
    return final
